# revision 1
# baseline (speedup 1.0000x reference)
"""DiffAttn (differential attention) Trainium2 Bass kernel.

Self-contained: kernel(**inputs) takes the FULL unsharded inputs as numpy
arrays and returns the FULL output [2, 4096, 128] float32.

Sharding: 8 cores = (batch in {0,1}) x (query-block of 1024 rows).
Each core projects Q, and K/V for only its OWN 1024-key block (the key
block is chosen equal to the query block, so a single per-core xq upload
feeds all three projections); the four cores sharing a batch then
AllGather the kT/V parts on-device, and each core runs the two
transposed-score softmaxes, the combined PV, and RMSNorm for its queries.

Layout strategy (the load-bearing decision): scores are computed
TRANSPOSED ([sk, sq], keys on partitions) so that exp(scores) can feed the
PV matmul directly as the streaming operand with V as stationary weights --
attention always contracts over sk, so the probability matrix must have sk
on partitions; producing it there directly avoids a PE transpose of the
full [sq, 4096] probability matrix per q-tile. Softmax row-sums are
recovered with a ones-stationary matmul, transposed back to per-partition
form (tiny [1,128] PE transposes) for the normalization, which happens
AFTER PV:   attn = U1/s1 - lam * U2/s2.
"""

import math
import os
import sys
from contextlib import ExitStack

import numpy as np

for _p in ("/root/.axon_site/_ro/trn_rl_repo", "/opt/trn_rl_repo"):
    if os.path.isdir(_p) and _p not in sys.path:
        sys.path.append(_p)

import ml_dtypes  # noqa: E402

import concourse.bass as bass  # noqa: E402
import concourse.mybir as mybir  # noqa: E402
import concourse.tile as tile  # noqa: E402
from concourse import bacc, bass_utils  # noqa: E402
from concourse.masks import make_identity  # noqa: E402

B, S, D, H = 2, 4096, 2048, 128
H2 = H // 2  # 64
P = 128
NCORES = 8
QSHARD = 1024  # q rows per core
DCH = D // P  # 16 d-chunks
NKCH = S // P  # 32 key chunks of 128
NGROUPS, GW = 2, 512  # q groups per core
NBLK, BLKW = 4, 1024  # key blocks for projections
NJ = GW // P  # 4 q sub-blocks of 128 per group

LAMBDA_INIT = 0.8 - 0.6 * math.exp(-0.3 * 12)
RMS_EPS = float(np.finfo(np.float32).eps)
SCALE = 1.0 / math.sqrt(H2)

F32 = mybir.dt.float32
BF16 = mybir.dt.bfloat16

AF = mybir.ActivationFunctionType
OP = mybir.AluOpType


def _emit(ctx: ExitStack, tc: "tile.TileContext", lam: float):
    nc = tc.nc

    # Each core projects K/V only for its own 1024-key block (== its q block,
    # so the single xq input feeds q, k and v projections), then the four
    # cores sharing a batch AllGather the kT/V parts.
    xq = nc.dram_tensor("xq", (D, QSHARD), BF16, kind="ExternalInput").ap()
    part_d = nc.dram_tensor("part_d", (2, P, BLKW), BF16).ap()
    full_d = nc.dram_tensor("full_d", (2 * NBLK, P, BLKW), BF16).ap()
    wqT = nc.dram_tensor("wqT", (D, H), BF16, kind="ExternalInput").ap()
    wkT = nc.dram_tensor("wkT", (D, H), BF16, kind="ExternalInput").ap()
    wvT = nc.dram_tensor("wvT", (D, H), BF16, kind="ExternalInput").ap()
    rmsw = nc.dram_tensor("rmsw", (H,), F32, kind="ExternalInput").ap()
    out_d = nc.dram_tensor("out", (QSHARD, H), F32, kind="ExternalOutput").ap()

    # ---- constant / persistent SBUF tiles ----
    consts = ctx.enter_context(tc.tile_pool(name="consts", bufs=1))
    persist = ctx.enter_context(tc.tile_pool(name="persist", bufs=1))

    ident = consts.tile([P, P], F32)
    make_identity(nc, ident)
    ones_bf = consts.tile([P, 1], BF16)
    nc.vector.memset(ones_bf, 1.0)
    rmsw_bc = consts.tile([P, H], F32)
    nc.sync.dma_start(
        out=rmsw_bc,
        in_=bass.AP(tensor=rmsw.tensor, offset=0, ap=[[0, P], [1, H]]),
    )
    # weight tiles: w_sb[p, c, h] = W?T[c*128 + p, h]; DMAs are issued in
    # first-use order further below (wk -> xq head -> wv -> xq tail -> wq) to
    # pull the collective dispatch as early as possible
    wq_sb = consts.tile([P, DCH, H], BF16)
    wk_sb = consts.tile([P, DCH, H], BF16)
    wv_sb = consts.tile([P, DCH, H], BF16)

    qT_sb = persist.tile([P, QSHARD], BF16)  # [h, sq]
    kT_sb = persist.tile([P, S], BF16)  # [h, sk]
    v_sb = persist.tile([P, NKCH, P], BF16)  # [sk%128, chunk, h]

    xpool = ctx.enter_context(tc.tile_pool(name="xstream", bufs=1))
    epool = ctx.enter_context(tc.tile_pool(name="epool", bufs=6))
    usb_pool = ctx.enter_context(tc.tile_pool(name="usb", bufs=2))
    small = ctx.enter_context(tc.tile_pool(name="small", bufs=8))
    outp = ctx.enter_context(tc.tile_pool(name="outp", bufs=4))
    attn_pool = ctx.enter_context(tc.tile_pool(name="attnp", bufs=2 * NGROUPS * NJ + 1))

    # ---- load xq once; project q, and this core's own-block kT/V ----
    xq_r = xq.rearrange("(c p) q -> p c q", p=P)
    xq_sb = xpool.tile([P, DCH, QSHARD], BF16, tag="xq", bufs=1)
    nc.sync.dma_start(out=wk_sb, in_=wkT.rearrange("(c p) h -> p c h", p=P))
    for c4 in range(4):
        nc.sync.dma_start(out=xq_sb[:, c4, :], in_=xq_r[:, c4, :])
    nc.sync.dma_start(out=wv_sb, in_=wvT.rearrange("(c p) h -> p c h", p=P))
    for qt in range(1, 4):
        nc.sync.dma_start(
            out=xq_sb[:, qt * 4 : (qt + 1) * 4, :], in_=xq_r[:, qt * 4 : (qt + 1) * 4, :]
        )
    nc.sync.dma_start(out=wq_sb, in_=wqT.rearrange("(c p) h -> p c h", p=P))

    kpart_sb = persist.tile([P, BLKW], BF16)
    vpart_sb = persist.tile([P, 8, P], BF16)
    # projection PSUM pools live only until the collective is dispatched
    with tc.tile_pool(name="pp_proj", space="PSUM", bufs=1) as pp_proj:
        for sl in range(2):
            kacc = pp_proj.tile([P, 512], F32, tag="kacc", bufs=2)
            for c in range(DCH):
                nc.tensor.matmul(
                    kacc,
                    wk_sb[:, c, :],
                    xq_sb[:, c, sl * 512 : (sl + 1) * 512],
                    start=(c == 0),
                    stop=(c == DCH - 1),
                )
            nc.scalar.copy(kpart_sb[:, sl * 512 : (sl + 1) * 512], kacc)
        # V natural layout, 4 subtiles at a time (one PSUM bank); accumulation
        # groups sharing a bank must not overlap, hence j-outer c-inner
        for hf in range(2):
            vacc = pp_proj.tile([P, 4, P], F32, tag="vacc", bufs=2)
            for j4 in range(4):
                j = hf * 4 + j4
                for c in range(DCH):
                    nc.tensor.matmul(
                        vacc[:, j4, :],
                        xq_sb[:, c, j * P : (j + 1) * P],
                        wv_sb[:, c, :],
                        start=(c == 0),
                        stop=(c == DCH - 1),
                    )
            nc.vector.tensor_copy(vpart_sb[:, hf * 4 : (hf + 1) * 4, :], vacc)

        # ---- AllGather kT/V parts across the 4 cores sharing a batch ----
        nc.sync.dma_start(out=part_d[0], in_=kpart_sb)
        nc.sync.dma_start(out=part_d[1], in_=vpart_sb.rearrange("p j h -> p (j h)"))
        nc.gpsimd.collective_compute(
            "AllGather",
            OP.bypass,
            replica_groups=[[0, 1, 2, 3], [4, 5, 6, 7]],
            ins=[part_d.opt()],
            outs=[full_d.opt()],
        )

        # qT projection overlaps the collective flight time
        for sl in range(2):
            qacc = pp_proj.tile([P, 512], F32, tag="kacc", bufs=2)
            for c in range(DCH):
                nc.tensor.matmul(
                    qacc,
                    wq_sb[:, c, :],
                    xq_sb[:, c, sl * 512 : (sl + 1) * 512],
                    start=(c == 0),
                    stop=(c == DCH - 1),
                )
            nc.scalar.copy(qT_sb[:, sl * 512 : (sl + 1) * 512], qacc)

    for r in range(NBLK):
        nc.sync.dma_start(out=kT_sb[:, r * BLKW : (r + 1) * BLKW], in_=full_d[2 * r])
        nc.sync.dma_start(
            out=v_sb[:, r * 8 : (r + 1) * 8, :],
            in_=full_d[2 * r + 1].rearrange("p (j h) -> p j h", j=8),
        )

    # attention PSUM pools (after proj pools close): s 4 + u 2 + sums 2 = 8
    pp_s = ctx.enter_context(tc.tile_pool(name="pp_s", space="PSUM", bufs=2))
    pp_u = ctx.enter_context(tc.tile_pool(name="pp_u", space="PSUM", bufs=1))
    pp_sum = ctx.enter_context(tc.tile_pool(name="pp_sum", space="PSUM", bufs=1))

    def attend_chunk(g, u_ps, sums_ps, kT_ap, v_ap, start, stop):
        """scores -> exp -> sums/U accumulation for one 128-key chunk."""
        q0 = g * GW
        s_ps = pp_s.tile([P, 2 * GW], F32, tag="s", name="s_ps")
        nc.tensor.matmul(s_ps[:, 0:GW], kT_ap[0:H2, :], qT_sb[0:H2, q0 : q0 + GW])
        nc.tensor.matmul(
            s_ps[:, GW : 2 * GW], kT_ap[H2:H, :], qT_sb[H2:H, q0 : q0 + GW]
        )
        e_sb = epool.tile([P, 2 * GW], BF16, tag="e", name="e_sb")
        nc.scalar.activation(e_sb, s_ps, AF.Exp, scale=SCALE)
        for hf in range(2):
            sl = slice(hf * GW, (hf + 1) * GW)
            nc.tensor.matmul(
                sums_ps[g * 32 : g * 32 + 1, sl],
                ones_bf,
                e_sb[:, sl],
                start=start,
                stop=stop,
            )
            nc.tensor.matmul(u_ps[:, sl], v_ap, e_sb[:, sl], start=start, stop=stop)

    u_sbs = []
    sums_acc = small.tile([1, NGROUPS, 2 * GW], F32, tag="sums_acc", bufs=1)
    sums_ps = pp_sum.tile([33, 2 * GW], F32, tag="sum")

    # ---- warm-up pass while the AllGather is in flight: run group 0's
    # attention against this core's OWN locally-projected key block. The
    # gathered pass below covers every block exactly once, so these results
    # are discarded -- the point is to keep PE/ACT busy (and the PE HAM
    # clock-gate warm) instead of idling through the collective.
    u_warm = pp_u.tile([P, 2 * GW], F32, tag="u", name="u_warm")
    for wi in range(16):
        g, c8 = divmod(wi, 8)
        q0 = g * GW
        s_ps = pp_s.tile([P, 2 * GW], F32, tag="s", name="s_warm")
        nc.tensor.matmul(
            s_ps[:, 0:GW],
            kpart_sb[0:H2, c8 * P : (c8 + 1) * P],
            qT_sb[0:H2, q0 : q0 + GW],
        )
        nc.tensor.matmul(
            s_ps[:, GW : 2 * GW],
            kpart_sb[H2:H, c8 * P : (c8 + 1) * P],
            qT_sb[H2:H, q0 : q0 + GW],
        )
        e_sb = epool.tile([P, 2 * GW], BF16, tag="e", name="e_warm")
        nc.scalar.activation(e_sb, s_ps, AF.Exp, scale=SCALE)
        for hf in range(2):
            sl = slice(hf * GW, (hf + 1) * GW)
            nc.tensor.matmul(
                u_warm[:, sl],
                vpart_sb[:, c8, :],
                e_sb[:, sl],
                start=(wi == 0),
                stop=(wi == 15),
            )

    # ---- the real attention: all four gathered blocks, per group ----
    for g in range(NGROUPS):
        u_ps = pp_u.tile([P, 2 * GW], F32, tag="u", name=f"u_ps{g}")
        for ch in range(NKCH):
            attend_chunk(
                g,
                u_ps,
                sums_ps,
                kT_sb[:, ch * P : (ch + 1) * P],
                v_sb[:, ch, :],
                start=(ch == 0),
                stop=(ch == NKCH - 1),
            )
        u_sb = usb_pool.tile([P, 2 * GW], F32, tag="usb")
        nc.vector.tensor_copy(u_sb, u_ps)
        nc.vector.tensor_copy(sums_acc[0:1, g, :], sums_ps[g * 32 : g * 32 + 1, :])
        u_sbs.append(u_sb)

    # ---- post phase: normalize + combine + RMS stats ----
    finals = []  # (attn_sb, rmsin_sb, row0)

    c_ = 1.0 - LAMBDA_INIT
    a_ = 1.0 / (H * c_ * c_)
    b_ = RMS_EPS / (c_ * c_)
    r_sb = small.tile([P, 2 * 2 * NJ], F32, tag="r", bufs=1)
    for g in range(NGROUPS):
        # sums -> per-partition layout via tiny PE transposes ("s"-tag psum
        # slots cycle quickly, letting group 0's post overlap group 1's tail)
        sumsT_ps = pp_s.tile([P, 2 * NJ], F32, tag="s")
        for hf in range(2):
            for j in range(NJ):
                nc.tensor.transpose(
                    sumsT_ps[:, hf * NJ + j : hf * NJ + j + 1],
                    sums_acc[0:1, g, hf * GW + j * P : hf * GW + (j + 1) * P],
                    ident[0:1, 0:1],
                )
        rg = r_sb[:, g * 2 * NJ : (g + 1) * 2 * NJ]
        nc.vector.reciprocal(rg, sumsT_ps)
        nc.vector.tensor_scalar_mul(
            r_sb[:, g * 2 * NJ + NJ : (g + 1) * 2 * NJ],
            r_sb[:, g * 2 * NJ + NJ : (g + 1) * 2 * NJ],
            lam,
        )

    for g in range(NGROUPS):
        post_ps = pp_u.tile([P, 2 * NJ, P], F32, tag="u")
        for j in range(NJ):
            nc.tensor.transpose(
                post_ps[:, j, :], u_sbs[g][:, j * P : (j + 1) * P], ident
            )
            nc.tensor.transpose(
                post_ps[:, NJ + j, :], u_sbs[g][:, GW + j * P : GW + (j + 1) * P], ident
            )
        for j in range(NJ):
            rcol = g * 2 * NJ
            t2 = small.tile([P, P], F32, tag="t2")
            nc.scalar.activation(
                t2,
                post_ps[:, NJ + j, :],
                AF.Copy,
                scale=r_sb[:, rcol + NJ + j : rcol + NJ + j + 1],
            )
            attn_sb = attn_pool.tile([P, P], F32, tag="attn")
            nc.vector.scalar_tensor_tensor(
                attn_sb,
                post_ps[:, j, :],
                r_sb[:, rcol + j : rcol + j + 1],
                t2,
                op0=OP.mult,
                op1=OP.subtract,
            )
            sq_scr = small.tile([P, P], F32, tag="sqscr")
            ssq = small.tile([P, 1], F32, tag="ssq")
            nc.scalar.activation(sq_scr, attn_sb, AF.Square, accum_out=ssq)
            rmsin = small.tile([P, 1], F32, tag="rmsin")
            nc.vector.tensor_scalar(rmsin, ssq, a_, b_, op0=OP.mult, op1=OP.add)
            finals.append((attn_sb, rmsin, g * GW + j * P))

    # ---- phase C: final normalization + store ----
    for attn_sb, rmsin, row0 in finals:
        root = small.tile([P, 1], F32, tag="root")
        nc.scalar.activation(root, rmsin, AF.Sqrt)
        rrms = small.tile([P, 1], F32, tag="rrms")
        nc.vector.reciprocal(rrms, root)
        o_sb = outp.tile([P, H], F32, tag="o")
        nc.vector.scalar_tensor_tensor(
            o_sb, attn_sb, rrms, rmsw_bc, op0=OP.mult, op1=OP.mult
        )
        nc.sync.dma_start(out=out_d[row0 : row0 + P, :], in_=o_sb)


def build(lam: float):
    from concourse._compat import axon_active

    # The axon/PJRT redirect path has no BassDebugger (no /dev/neuron* on the
    # client), so it needs debug=False; the native NrtSession path expects a
    # debug-enabled Bass (mirrors bass_test_utils.run_kernel).
    nc = bacc.Bacc(
        "TRN2",
        target_bir_lowering=False,
        debug=not axon_active(),
        num_devices=NCORES,
    )
    with tile.TileContext(nc) as tc:
        with ExitStack() as ctx:
            _emit(ctx, tc, lam)
    nc.compile()
    return nc


def make_in_maps(x, Wq, Wk, Wv, rms_weight):
    bf = ml_dtypes.bfloat16
    x = np.asarray(x, dtype=np.float32)
    xT = np.ascontiguousarray(x.transpose(0, 2, 1)).astype(bf)  # [B, D, S]
    wqT = np.ascontiguousarray(np.asarray(Wq, np.float32).T).astype(bf)
    wkT = np.ascontiguousarray(np.asarray(Wk, np.float32).T).astype(bf)
    wvT = np.ascontiguousarray(np.asarray(Wv, np.float32).T).astype(bf)
    rw = np.ascontiguousarray(np.asarray(rms_weight, np.float32))
    in_maps = []
    for core in range(NCORES):
        b, qb = divmod(core, NCORES // B)
        in_maps.append(
            {
                "xq": np.ascontiguousarray(xT[b][:, qb * QSHARD : (qb + 1) * QSHARD]),
                "wqT": wqT,
                "wkT": wkT,
                "wvT": wvT,
                "rmsw": rw,
            }
        )
    return in_maps


def kernel(x, Wq, Wk, Wv, lambda_q1, lambda_q2, lambda_k1, lambda_k2, rms_weight):
    lq1 = np.asarray(lambda_q1, np.float32)
    lq2 = np.asarray(lambda_q2, np.float32)
    lk1 = np.asarray(lambda_k1, np.float32)
    lk2 = np.asarray(lambda_k2, np.float32)
    lam = float(
        np.exp(np.dot(lq1, lk1)) - np.exp(np.dot(lq2, lk2)) + LAMBDA_INIT
    )
    nc = build(lam)
    in_maps = make_in_maps(x, Wq, Wk, Wv, rms_weight)
    res = bass_utils.run_bass_kernel_spmd(nc, in_maps, core_ids=list(range(NCORES)))
    out = np.empty((B, S, H), np.float32)
    for core in range(NCORES):
        b, qb = divmod(core, NCORES // B)
        out[b, qb * QSHARD : (qb + 1) * QSHARD] = res.results[core]["out"]
    return out



# revision 9
# speedup vs baseline: 1.5354x; 1.5354x over previous
"""DiffAttn (differential attention) Trainium2 Bass kernel, v2.

Self-contained: kernel(**inputs) takes the FULL unsharded inputs as numpy
arrays and returns the FULL output [2, 4096, 128] float32.

Sharding: 8 cores = (batch in {0,1}) x (query-block of 1024 rows).  Key
blocks are consumed in XOR-delta order (softmax over keys is permutation
invariant, so block order is irrelevant): the host stages each core's x
slices pre-permuted as [own, own^1, own^2], the core projects K/V for those
three blocks locally, and block own^3 is exchanged through a tiny PAIRWISE
AllGather ([[0,3],[1,2],[4,7],[5,6]]) whose output holds {own, own^3} in
rank order -- both halves are attended (own is NOT attended from the local
projection), which keeps the program identity-free SPMD.

Scores are computed TRANSPOSED ([sk, sq], keys on partitions) so exp(scores)
feeds the PV matmul directly as the moving operand (attention contracts over
sk).  Softmax row-sums are NOT computed with per-chunk ones-matmuls (that
costs as much PE as the PV itself): instead the DVE accumulates e chunk-wise
into an fp16 running sum tile and a single ones-matmul per group recovers
the sums at the end of the sweep.  exp is biased by -EXP_BIAS (cancels in
u/sums) so the fp16 accumulation cannot overflow.

The main loop is ACT(exp)-bound; emission is software-pipelined depth-1
(scores(i+1) issues before PV(i)) which keeps the ACT engine saturated.
"""

import math
import os
import sys
from contextlib import ExitStack

import numpy as np

for _p in ("/root/.axon_site/_ro/trn_rl_repo", "/opt/trn_rl_repo"):
    if os.path.isdir(_p) and _p not in sys.path:
        sys.path.append(_p)

import ml_dtypes  # noqa: E402

import concourse.bass as bass  # noqa: E402
import concourse.mybir as mybir  # noqa: E402
import concourse.tile as tile  # noqa: E402
from concourse import bacc, bass_utils  # noqa: E402
from concourse.masks import make_identity  # noqa: E402

B, S, D, H = 2, 4096, 2048, 128
H2 = H // 2  # 64
P = 128
NCORES = 8
QSHARD = 1024  # q rows per core
DCH = D // P  # 16 d-chunks
NGROUPS, GW = 2, 512  # q groups per core
NBLK, BLKW = 4, 1024  # attended key blocks per core
BCH = BLKW // P  # 8 key chunks per block
NJ = GW // P  # 4 q sub-blocks of 128 per group

LAMBDA_INIT = 0.8 - 0.6 * math.exp(-0.3 * 12)
RMS_EPS = float(np.finfo(np.float32).eps)
SCALE = 1.0 / math.sqrt(H2)
EXP_BIAS = 4.0  # exp(s*SCALE - EXP_BIAS); cancels in u/sums

F32 = mybir.dt.float32
BF16 = mybir.dt.bfloat16
F16 = mybir.dt.float16

AF = mybir.ActivationFunctionType
OP = mybir.AluOpType


def _emit(ctx: ExitStack, tc: "tile.TileContext", lam: float):
    nc = tc.nc

    # x slices for blocks [own, own^1, own^2] (host pre-permutes per core)
    xb = nc.dram_tensor("xb", (D, 3, BLKW), BF16, kind="ExternalInput").ap()
    part_d = nc.dram_tensor("part_d", (2, P, BLKW), F16).ap()
    pair_d = nc.dram_tensor("pair_d", (4, P, BLKW), F16).ap()
    wqT = nc.dram_tensor("wqT", (D, H), BF16, kind="ExternalInput").ap()
    wkT = nc.dram_tensor("wkT", (D, H), BF16, kind="ExternalInput").ap()
    wvT = nc.dram_tensor("wvT", (D, H), BF16, kind="ExternalInput").ap()
    rmsw = nc.dram_tensor("rmsw", (H,), F32, kind="ExternalInput").ap()
    out_d = nc.dram_tensor("out", (QSHARD, H), F32, kind="ExternalOutput").ap()

    # ---- constant / persistent SBUF tiles ----
    consts = ctx.enter_context(tc.tile_pool(name="consts", bufs=1))
    persist = ctx.enter_context(tc.tile_pool(name="persist", bufs=1))

    ident = consts.tile([P, P], F32)
    make_identity(nc, ident)
    ones_f16 = consts.tile([P, 1], F16)
    nc.vector.memset(ones_f16, 1.0)
    nbias = consts.tile([P, 1], F32)
    nc.vector.memset(nbias, -EXP_BIAS)
    rmsw_bc = consts.tile([P, H], F32)
    nc.sync.dma_start(
        out=rmsw_bc,
        in_=bass.AP(tensor=rmsw.tensor, offset=0, ap=[[0, P], [1, H]]),
    )
    wq_sb = consts.tile([P, DCH, H], BF16)
    wk_sb = consts.tile([P, DCH, H], BF16)
    wv_sb = consts.tile([P, DCH, H], BF16)

    qT_sb = persist.tile([P, QSHARD], F16)  # [h, sq]
    # attended key blocks: slot 0 = own^1, 1 = own^2, 2/3 = pair {own, own^3}
    kT_sb = persist.tile([P, NBLK, BLKW], F16)  # [h, blk, sk]
    v_sb = persist.tile([P, NBLK, BCH, P], F16)  # [sk%128, blk, chunk, h]
    kpart_sb = persist.tile([P, BLKW], F16)  # own k (collective input only)
    vpart_sb = persist.tile([P, BCH, P], F16)  # own v (collective input only)

    xpool = ctx.enter_context(tc.tile_pool(name="xstream", bufs=3))
    epool = ctx.enter_context(tc.tile_pool(name="epool", bufs=4))
    accp = ctx.enter_context(tc.tile_pool(name="accp", bufs=1))
    usb_pool = ctx.enter_context(tc.tile_pool(name="usb", bufs=2))
    small = ctx.enter_context(tc.tile_pool(name="small", bufs=8))
    outp = ctx.enter_context(tc.tile_pool(name="outp", bufs=4))
    attn_pool = ctx.enter_context(tc.tile_pool(name="attnp", bufs=2 * NGROUPS * NJ + 1))

    xb_r = xb.rearrange("(c p) t q -> p c t q", p=P)

    # weight DMAs interleaved with first x block for earliest k-proj start
    nc.sync.dma_start(out=wk_sb, in_=wkT.rearrange("(c p) h -> p c h", p=P))
    x_tiles = []
    for t in range(3):
        x_sb = xpool.tile([P, DCH, BLKW], BF16, tag="x", name=f"x{t}")
        if t == 0:
            for c4 in range(4):
                nc.sync.dma_start(
                    out=x_sb[:, c4 * 4 : (c4 + 1) * 4, :],
                    in_=xb_r[:, c4 * 4 : (c4 + 1) * 4, 0, :],
                )
            nc.sync.dma_start(out=wv_sb, in_=wvT.rearrange("(c p) h -> p c h", p=P))
            nc.sync.dma_start(out=wq_sb, in_=wqT.rearrange("(c p) h -> p c h", p=P))
        else:
            nc.sync.dma_start(out=x_sb, in_=xb_r[:, :, t, :])
        x_tiles.append(x_sb)

    def proj_k(x_sb, dst):  # dst [P, BLKW] f16 slice
        for sl in range(2):
            kacc = pp_proj.tile([P, GW], F32, tag="kacc", bufs=2)
            for c in range(DCH):
                nc.tensor.matmul(
                    kacc,
                    wk_sb[:, c, :],
                    x_sb[:, c, sl * GW : (sl + 1) * GW],
                    start=(c == 0),
                    stop=(c == DCH - 1),
                )
            nc.scalar.copy(dst[:, sl * GW : (sl + 1) * GW], kacc)

    def proj_v(x_sb, dst):  # dst [P, BCH, P] f16 slice
        for hf in range(2):
            vacc = pp_proj.tile([P, 4, P], F32, tag="vacc", bufs=2)
            for j4 in range(4):
                j = hf * 4 + j4
                for c in range(DCH):
                    nc.tensor.matmul(
                        vacc[:, j4, :],
                        x_sb[:, c, j * P : (j + 1) * P],
                        wv_sb[:, c, :],
                        start=(c == 0),
                        stop=(c == DCH - 1),
                    )
            nc.vector.tensor_copy(dst[:, hf * 4 : (hf + 1) * 4, :], vacc)

    with tc.tile_pool(name="pp_proj", space="PSUM", bufs=1) as pp_proj:
        # own block: project K/V, ship through the pairwise collective
        proj_k(x_tiles[0], kpart_sb)
        proj_v(x_tiles[0], vpart_sb)
        nc.sync.dma_start(out=part_d[0], in_=kpart_sb)
        nc.sync.dma_start(out=part_d[1], in_=vpart_sb.rearrange("p j h -> p (j h)"))
        nc.gpsimd.collective_compute(
            "AllGather",
            OP.bypass,
            replica_groups=[[0, 3], [1, 2], [4, 7], [5, 6]],
            ins=[part_d.opt()],
            outs=[pair_d.opt()],
        )
        # q projection + d1/d2 K/V projections overlap the collective flight
        for sl in range(2):
            qacc = pp_proj.tile([P, GW], F32, tag="kacc", bufs=2)
            for c in range(DCH):
                nc.tensor.matmul(
                    qacc,
                    wq_sb[:, c, :],
                    x_tiles[0][:, c, sl * GW : (sl + 1) * GW],
                    start=(c == 0),
                    stop=(c == DCH - 1),
                )
            nc.scalar.copy(qT_sb[:, sl * GW : (sl + 1) * GW], qacc)
        for t in (1, 2):
            proj_k(x_tiles[t], kT_sb[:, t - 1, :])
            proj_v(x_tiles[t], v_sb[:, t - 1, :, :])

    # unpack the gathered pair {own, own^3} into slots 2, 3
    for r in range(2):
        nc.sync.dma_start(out=kT_sb[:, 2 + r, :], in_=pair_d[2 * r])
        nc.sync.dma_start(
            out=v_sb[:, 2 + r, :, :],
            in_=pair_d[2 * r + 1].rearrange("p (j h) -> p j h", j=BCH),
        )

    # ---- attention sweep: 4 blocks x 2 groups x 8 chunks, pipelined ----
    pp_s = ctx.enter_context(tc.tile_pool(name="pp_s", space="PSUM", bufs=2))
    pp_u = ctx.enter_context(tc.tile_pool(name="pp_u", space="PSUM", bufs=1))

    # u accumulators for both groups in one 4-bank tile; accumulation groups
    # for (g, hf) quadrants live in disjoint banks
    u_ps = pp_u.tile([P, NGROUPS, 2, GW], F32, tag="u", name="u_ps")
    acc = [accp.tile([P, 2 * GW], F16, tag=f"acc{g}", name=f"acc{g}") for g in range(2)]

    pending = None  # (e_sb, g, blk, ch) awaiting PV + DVE accumulation

    def flush_pending():
        nonlocal pending
        if pending is None:
            return
        e_sb, g, blk, ch = pending
        first = blk == 0 and ch == 0
        last = blk == NBLK - 1 and ch == BCH - 1
        for hf in range(2):
            nc.tensor.matmul(
                u_ps[:, g, hf, :],
                v_sb[:, blk, ch, :],
                e_sb[:, hf * GW : (hf + 1) * GW],
                start=first,
                stop=last,
            )
        if first:
            nc.vector.tensor_copy(acc[g], e_sb)
        else:
            nc.vector.tensor_tensor(acc[g], acc[g], e_sb, op=OP.add)
        pending = None

    for blk in range(NBLK):
        for g in range(NGROUPS):
            q0 = g * GW
            for ch in range(BCH):
                s_ps = pp_s.tile([P, 2 * GW], F32, tag="s", name="s_ps")
                k0 = blk  # slot
                nc.tensor.matmul(
                    s_ps[:, 0:GW],
                    kT_sb[0:H2, k0, ch * P : (ch + 1) * P],
                    qT_sb[0:H2, q0 : q0 + GW],
                )
                nc.tensor.matmul(
                    s_ps[:, GW : 2 * GW],
                    kT_sb[H2:H, k0, ch * P : (ch + 1) * P],
                    qT_sb[H2:H, q0 : q0 + GW],
                )
                e_sb = epool.tile([P, 2 * GW], F16, tag="e", name="e_sb")
                nc.scalar.activation(e_sb, s_ps, AF.Exp, scale=SCALE, bias=nbias)
                flush_pending()
                pending = (e_sb, g, blk, ch)
    flush_pending()

    # ---- group tails: u -> SBUF, sums via one ones-matmul per group ----
    u_sbs = []
    sums_acc = small.tile([1, NGROUPS, 2 * GW], F32, tag="sums_acc", bufs=1)
    for g in range(NGROUPS):
        u_sb = usb_pool.tile([P, 2 * GW], F32, tag="usb")
        nc.vector.tensor_copy(u_sb, u_ps[:, g, :, :])
        u_sbs.append(u_sb)
        sums_ps = pp_s.tile([P, 2 * GW], F32, tag="s", name=f"sums{g}")
        for hf in range(2):
            sl = slice(hf * GW, (hf + 1) * GW)
            nc.tensor.matmul(sums_ps[0:1, sl], ones_f16, acc[g][:, sl])
        nc.vector.tensor_copy(sums_acc[0:1, g, :], sums_ps[0:1, :])

    # ---- post phase: normalize + combine + RMS stats ----
    c_ = 1.0 - LAMBDA_INIT
    a_ = 1.0 / (H * c_ * c_)
    b_ = RMS_EPS / (c_ * c_)
    r_sb = small.tile([P, 2 * 2 * NJ], F32, tag="r", bufs=1)
    for g in range(NGROUPS):
        sumsT_ps = pp_s.tile([P, 2 * NJ], F32, tag="s")
        for hf in range(2):
            for j in range(NJ):
                nc.tensor.transpose(
                    sumsT_ps[:, hf * NJ + j : hf * NJ + j + 1],
                    sums_acc[0:1, g, hf * GW + j * P : hf * GW + (j + 1) * P],
                    ident[0:1, 0:1],
                )
        rg = r_sb[:, g * 2 * NJ : (g + 1) * 2 * NJ]
        nc.vector.reciprocal(rg, sumsT_ps)
        nc.vector.tensor_scalar_mul(
            r_sb[:, g * 2 * NJ + NJ : (g + 1) * 2 * NJ],
            r_sb[:, g * 2 * NJ + NJ : (g + 1) * 2 * NJ],
            lam,
        )

    rmsall = small.tile([P, NGROUPS * NJ], F32, tag="rmsall", bufs=1)
    attns = []  # (attn_sb, idx, row0)
    for g in range(NGROUPS):
        post_ps = pp_s.tile([P, 2 * NJ, P], F32, tag="s", name=f"post{g}")
        for j in range(NJ):
            nc.tensor.transpose(
                post_ps[:, j, :], u_sbs[g][:, j * P : (j + 1) * P], ident
            )
            nc.tensor.transpose(
                post_ps[:, NJ + j, :], u_sbs[g][:, GW + j * P : GW + (j + 1) * P], ident
            )
        for j in range(NJ):
            rcol = g * 2 * NJ
            idx = g * NJ + j
            t2 = small.tile([P, P], F32, tag="t2")
            nc.vector.scalar_tensor_tensor(
                t2,
                post_ps[:, NJ + j, :],
                r_sb[:, rcol + NJ + j : rcol + NJ + j + 1],
                rmsw_bc,
                op0=OP.mult,
                op1=OP.bypass,
            )
            attn_sb = attn_pool.tile([P, P], F32, tag="attn")
            nc.vector.scalar_tensor_tensor(
                attn_sb,
                post_ps[:, j, :],
                r_sb[:, rcol + j : rcol + j + 1],
                t2,
                op0=OP.mult,
                op1=OP.subtract,
            )
            sq_scr = small.tile([P, P], F32, tag="sqscr")
            nc.scalar.activation(
                sq_scr, attn_sb, AF.Square, accum_out=rmsall[:, idx : idx + 1]
            )
            attns.append((attn_sb, idx, g * GW + j * P))

    # batched RMS chain over all 8 row-tiles at once
    rmsf = small.tile([P, NGROUPS * NJ], F32, tag="rmsf", bufs=1)
    nc.vector.tensor_scalar(rmsf, rmsall, a_, b_, op0=OP.mult, op1=OP.add)
    roots = small.tile([P, NGROUPS * NJ], F32, tag="roots", bufs=1)
    nc.scalar.activation(roots, rmsf, AF.Sqrt)
    rrms_all = small.tile([P, NGROUPS * NJ], F32, tag="rrms", bufs=1)
    nc.vector.reciprocal(rrms_all, roots)
    o_all = outp.tile([P, NGROUPS * NJ, H], F32, tag="o", bufs=1)
    for attn_sb, idx, row0 in attns:
        nc.vector.scalar_tensor_tensor(
            o_all[:, idx, :],
            attn_sb,
            rrms_all[:, idx : idx + 1],
            rmsw_bc,
            op0=OP.mult,
            op1=OP.mult,
        )
    nc.sync.dma_start(out=out_d.rearrange("(j p) h -> p j h", p=P), in_=o_all)


def build(lam: float):
    from concourse._compat import axon_active

    nc = bacc.Bacc(
        "TRN2",
        target_bir_lowering=False,
        debug=not axon_active(),
        num_devices=NCORES,
    )
    with tile.TileContext(nc) as tc:
        with ExitStack() as ctx:
            _emit(ctx, tc, lam)
    nc.compile()
    return nc


def make_in_maps(x, Wq, Wk, Wv, rms_weight):
    bf = ml_dtypes.bfloat16
    x = np.asarray(x, dtype=np.float32)
    xT = np.ascontiguousarray(x.transpose(0, 2, 1)).astype(bf)  # [B, D, S]
    wqT = np.ascontiguousarray(np.asarray(Wq, np.float32).T).astype(bf)
    wkT = np.ascontiguousarray(np.asarray(Wk, np.float32).T).astype(bf)
    wvT = np.ascontiguousarray(np.asarray(Wv, np.float32).T).astype(bf)
    rw = np.ascontiguousarray(np.asarray(rms_weight, np.float32))
    in_maps = []
    for core in range(NCORES):
        b, qb = divmod(core, NCORES // B)
        xb = np.empty((D, 3, BLKW), bf)
        for t in range(3):
            rb = qb ^ t
            xb[:, t, :] = xT[b][:, rb * BLKW : (rb + 1) * BLKW]
        in_maps.append(
            {
                "xb": np.ascontiguousarray(xb),
                "wqT": wqT,
                "wkT": wkT,
                "wvT": wvT,
                "rmsw": rw,
            }
        )
    return in_maps


def kernel(x, Wq, Wk, Wv, lambda_q1, lambda_q2, lambda_k1, lambda_k2, rms_weight):
    lq1 = np.asarray(lambda_q1, np.float32)
    lq2 = np.asarray(lambda_q2, np.float32)
    lk1 = np.asarray(lambda_k1, np.float32)
    lk2 = np.asarray(lambda_k2, np.float32)
    lam = float(
        np.exp(np.dot(lq1, lk1)) - np.exp(np.dot(lq2, lk2)) + LAMBDA_INIT
    )
    nc = build(lam)
    in_maps = make_in_maps(x, Wq, Wk, Wv, rms_weight)
    res = bass_utils.run_bass_kernel_spmd(nc, in_maps, core_ids=list(range(NCORES)))
    out = np.empty((B, S, H), np.float32)
    for core in range(NCORES):
        b, qb = divmod(core, NCORES // B)
        out[b, qb * QSHARD : (qb + 1) * QSHARD] = res.results[core]["out"]
    return out


# revision 26
# speedup vs baseline: 1.5590x; 1.0154x over previous
"""DiffAttn (differential attention) Trainium2 Bass kernel, v2.

Self-contained: kernel(**inputs) takes the FULL unsharded inputs as numpy
arrays and returns the FULL output [2, 4096, 128] float32.

Sharding: 8 cores = (batch in {0,1}) x (query-block of 1024 rows).  Key
blocks are consumed in XOR-delta order (softmax over keys is permutation
invariant, so block order is irrelevant): the host stages each core's x
slices pre-permuted as [own, own^1, own^2], the core projects K/V for those
three blocks locally, and block own^3 is exchanged through a tiny PAIRWISE
AllGather ([[0,3],[1,2],[4,7],[5,6]]) whose output holds {own, own^3} in
rank order -- both halves are attended (own is NOT attended from the local
projection), which keeps the program identity-free SPMD.

Scores are computed TRANSPOSED ([sk, sq], keys on partitions) so exp(scores)
feeds the PV matmul directly as the moving operand (attention contracts over
sk).  Softmax row-sums are NOT computed with per-chunk ones-matmuls (that
costs as much PE as the PV itself): instead the DVE accumulates e chunk-wise
into an fp16 running sum tile and a single ones-matmul per group recovers
the sums at the end of the sweep.  exp is biased by -EXP_BIAS (cancels in
u/sums) so the fp16 accumulation cannot overflow.

The main loop is ACT(exp)-bound; emission is software-pipelined depth-1
(scores(i+1) issues before PV(i)) which keeps the ACT engine saturated.
"""

import math
import os
import sys
from contextlib import ExitStack

import numpy as np

for _p in ("/root/.axon_site/_ro/trn_rl_repo", "/opt/trn_rl_repo"):
    if os.path.isdir(_p) and _p not in sys.path:
        sys.path.append(_p)

import ml_dtypes  # noqa: E402

import concourse.bass as bass  # noqa: E402
import concourse.mybir as mybir  # noqa: E402
import concourse.tile as tile  # noqa: E402
from concourse import bacc, bass_utils  # noqa: E402
from concourse.masks import make_identity  # noqa: E402

B, S, D, H = 2, 4096, 2048, 128
H2 = H // 2  # 64
P = 128
NCORES = 8
QSHARD = 1024  # q rows per core
DCH = D // P  # 16 d-chunks
NGROUPS, GW = 2, 512  # q groups per core
NBLK, BLKW = 4, 1024  # attended key blocks per core
BCH = BLKW // P  # 8 key chunks per block
NJ = GW // P  # 4 q sub-blocks of 128 per group

LAMBDA_INIT = 0.8 - 0.6 * math.exp(-0.3 * 12)
RMS_EPS = float(np.finfo(np.float32).eps)
SCALE = 1.0 / math.sqrt(H2)
EXP_BIAS = 4.0  # exp(s*SCALE - EXP_BIAS); cancels in u/sums

F32 = mybir.dt.float32
BF16 = mybir.dt.bfloat16
F16 = mybir.dt.float16

AF = mybir.ActivationFunctionType
OP = mybir.AluOpType


def _emit(ctx: ExitStack, tc: "tile.TileContext", lam: float):
    nc = tc.nc

    # x slices for blocks [own, own^1, own^2] (host pre-permutes per core)
    xb = nc.dram_tensor("xb", (D, 3, BLKW), BF16, kind="ExternalInput").ap()
    part_d = nc.dram_tensor("part_d", (2, P, BLKW), F16).ap()
    pair_d = nc.dram_tensor("pair_d", (4, P, BLKW), F16).ap()
    wqT = nc.dram_tensor("wqT", (D, H), BF16, kind="ExternalInput").ap()
    wkT = nc.dram_tensor("wkT", (D, H), BF16, kind="ExternalInput").ap()
    wvT = nc.dram_tensor("wvT", (D, H), BF16, kind="ExternalInput").ap()
    rmsw = nc.dram_tensor("rmsw", (H,), F32, kind="ExternalInput").ap()
    out_d = nc.dram_tensor("out", (QSHARD, H), F32, kind="ExternalOutput").ap()

    # ---- constant / persistent SBUF tiles ----
    consts = ctx.enter_context(tc.tile_pool(name="consts", bufs=1))
    persist = ctx.enter_context(tc.tile_pool(name="persist", bufs=1))

    ident = consts.tile([P, P], F32)
    make_identity(nc, ident)
    ident16 = consts.tile([P, P], F16)
    make_identity(nc, ident16)
    ones_f16 = consts.tile([P, 1], F16)
    nc.vector.memset(ones_f16, 1.0)
    nbias = consts.tile([P, 1], F32)
    nc.vector.memset(nbias, -EXP_BIAS)
    rmsw_bc = consts.tile([P, H], F32)
    nc.sync.dma_start(
        out=rmsw_bc,
        in_=bass.AP(tensor=rmsw.tensor, offset=0, ap=[[0, P], [1, H]]),
    )
    wq_sb = consts.tile([P, DCH, H], BF16)
    wk_sb = consts.tile([P, DCH, H], BF16)
    wv_sb = consts.tile([P, DCH, H], BF16)

    qT_sb = persist.tile([P, QSHARD], F16)  # [h, sq]
    # attended key blocks: slot 0 = own^1, 1 = own^2, 2/3 = pair {own, own^3}
    kT_sb = persist.tile([P, NBLK, BLKW], F16)  # [h, blk, sk]
    v_sb = persist.tile([P, NBLK, BCH, P], F16)  # [sk%128, blk, chunk, h]
    kpart_sb = persist.tile([P, BLKW], F16)  # own k (collective input only)
    vpart_sb = persist.tile([P, BCH, P], F16)  # own v (collective input only)

    xpool = ctx.enter_context(tc.tile_pool(name="xstream", bufs=3))
    epool = ctx.enter_context(tc.tile_pool(name="epool", bufs=4))
    accp = ctx.enter_context(tc.tile_pool(name="accp", bufs=1))
    usb_pool = ctx.enter_context(tc.tile_pool(name="usb", bufs=2))
    small = ctx.enter_context(tc.tile_pool(name="small", bufs=8))
    outp = ctx.enter_context(tc.tile_pool(name="outp", bufs=4))
    attn_pool = ctx.enter_context(tc.tile_pool(name="attnp", bufs=2 * NGROUPS * NJ + 1))

    xb_r = xb.rearrange("(c p) t q -> p c t q", p=P)

    # x piece-wise uploads so the c-outer projections chase the DMA
    nc.sync.dma_start(out=wk_sb, in_=wkT.rearrange("(c p) h -> p c h", p=P))
    x_tiles = []
    for t in range(3):
        x_sb = xpool.tile([P, DCH, BLKW], BF16, tag="x", name=f"x{t}")
        npc = 8 if t == 0 else 4
        step = DCH // npc
        for pc in range(npc):
            nc.sync.dma_start(
                out=x_sb[:, pc * step : (pc + 1) * step, :],
                in_=xb_r[:, pc * step : (pc + 1) * step, t, :],
            )
            if t == 0 and pc == 0:
                nc.sync.dma_start(
                    out=wv_sb, in_=wvT.rearrange("(c p) h -> p c h", p=P)
                )
            if t == 0 and pc == 5:
                nc.sync.dma_start(
                    out=wq_sb, in_=wqT.rearrange("(c p) h -> p c h", p=P)
                )
        x_tiles.append(x_sb)

    vT_scr = persist.tile([P, BLKW], F16)  # scratch: vT before transposing

    def proj_kv(x_sb, kdst, vdst):
        """c-outer K + transposed-V projection (both chase the x pieces),
        then PE-transpose vT back to the natural [sk, h] layout."""
        ks = [
            pp_proj.tile([P, GW], F32, tag=f"kacc{i}", bufs=1, name=f"kacc{i}")
            for i in range(2)
        ]
        vs = [
            pp_proj.tile([P, GW], F32, tag=f"vacc{i}", bufs=1, name=f"vacc{i}")
            for i in range(2)
        ]
        for c in range(DCH):
            for sl in range(2):
                xs = x_sb[:, c, sl * GW : (sl + 1) * GW]
                nc.tensor.matmul(
                    ks[sl], wk_sb[:, c, :], xs, start=(c == 0), stop=(c == DCH - 1)
                )
                nc.tensor.matmul(
                    vs[sl], wv_sb[:, c, :], xs, start=(c == 0), stop=(c == DCH - 1)
                )
        for sl in range(2):
            nc.scalar.copy(kdst[:, sl * GW : (sl + 1) * GW], ks[sl])
            nc.scalar.copy(vT_scr[:, sl * GW : (sl + 1) * GW], vs[sl])
        for j in range(BCH):
            tp = pp_proj.tile([P, P], F16, tag="tp", bufs=2)
            nc.tensor.transpose(tp, vT_scr[:, j * P : (j + 1) * P], ident16)
            nc.vector.tensor_copy(vdst[:, j, :], tp)

    with tc.tile_pool(name="pp_proj", space="PSUM", bufs=1) as pp_proj:
        # PE p-state warm-up while the first x pieces are in flight: the
        # Tensor engine reaches full clock after ~3us of continuous work
        for w in range(26):
            wtp = pp_proj.tile([P, P], F16, tag="tp", bufs=2, name="warm")
            nc.tensor.transpose(wtp, ident16, ident16)
        # own block: project K/V, ship through the pairwise collective
        proj_kv(x_tiles[0], kpart_sb, vpart_sb)
        nc.sync.dma_start(out=part_d[0], in_=kpart_sb)
        nc.sync.dma_start(out=part_d[1], in_=vpart_sb.rearrange("p j h -> p (j h)"))
        nc.gpsimd.collective_compute(
            "AllGather",
            OP.bypass,
            replica_groups=[[0, 3], [1, 2], [4, 7], [5, 6]],
            ins=[part_d.opt()],
            outs=[pair_d.opt()],
        )
        # q projection + d1/d2 K/V projections overlap the collective flight
        for sl in range(2):
            qacc = pp_proj.tile([P, GW], F32, tag="kacc0", bufs=1)
            for c in range(DCH):
                nc.tensor.matmul(
                    qacc,
                    wq_sb[:, c, :],
                    x_tiles[0][:, c, sl * GW : (sl + 1) * GW],
                    start=(c == 0),
                    stop=(c == DCH - 1),
                )
            nc.scalar.copy(qT_sb[:, sl * GW : (sl + 1) * GW], qacc)
        for t in (1, 2):
            proj_kv(x_tiles[t], kT_sb[:, t - 1, :], v_sb[:, t - 1, :, :])

    # unpack the gathered pair {own, own^3} into slots 2, 3
    for r in range(2):
        nc.sync.dma_start(out=kT_sb[:, 2 + r, :], in_=pair_d[2 * r])
        nc.sync.dma_start(
            out=v_sb[:, 2 + r, :, :],
            in_=pair_d[2 * r + 1].rearrange("p (j h) -> p j h", j=BCH),
        )

    # ---- attention sweep: 4 blocks x 2 groups x 8 chunks, pipelined ----
    pp_s = ctx.enter_context(tc.tile_pool(name="pp_s", space="PSUM", bufs=2))
    pp_u = ctx.enter_context(tc.tile_pool(name="pp_u", space="PSUM", bufs=1))

    # per-group u accumulators; (g, hf) accumulation groups in disjoint banks
    u_ps = [
        pp_u.tile([P, 2, GW], F32, tag=f"u{g}", bufs=1, name=f"u{g}")
        for g in range(NGROUPS)
    ]
    acc = [accp.tile([P, 2 * GW], F16, tag=f"acc{g}", name=f"acc{g}") for g in range(2)]

    pending = None  # (e_sb, g, blk, ch) awaiting PV + DVE accumulation

    def flush_pending():
        nonlocal pending
        if pending is None:
            return
        e_sb, g, blk, ch = pending
        first = blk == 0 and ch == 0
        last = blk == NBLK - 1 and ch == BCH - 1
        for hf in range(2):
            nc.tensor.matmul(
                u_ps[g][:, hf, :],
                v_sb[:, blk, ch, :],
                e_sb[:, hf * GW : (hf + 1) * GW],
                start=first,
                stop=last,
            )
        if first:
            nc.vector.tensor_copy(acc[g], e_sb)
        else:
            nc.vector.tensor_tensor(acc[g], acc[g], e_sb, op=OP.add)
        pending = None

    c_ = 1.0 - LAMBDA_INIT
    a_ = 1.0 / (H * c_ * c_)
    b_ = RMS_EPS / (c_ * c_)
    sums_acc = small.tile([1, NGROUPS, 2 * GW], F32, tag="sums_acc", bufs=1)
    r_sb = small.tile([P, 2 * 2 * NJ], F32, tag="r", bufs=1)
    rmsall = small.tile([P, NGROUPS * NJ], F32, tag="rmsall", bufs=1)
    u_sbs = {}

    def group_tail(g):
        """u -> SBUF, sums, normalization factors, combine, RMS, store —
        emitted right after group g's final PV so g0's tail overlaps g1's
        last block of attention."""
        u_sb = usb_pool.tile([P, 2 * GW], F32, tag="usb")
        nc.vector.tensor_copy(u_sb, u_ps[g][:, :, :])
        u_sbs[g] = u_sb
        sums_ps = pp_u.tile([P, 2 * GW], F32, tag=f"u{g}", bufs=1, name=f"sums{g}")
        for hf in range(2):
            sl = slice(hf * GW, (hf + 1) * GW)
            nc.tensor.matmul(sums_ps[0:1, sl], ones_f16, acc[g][:, sl])
        nc.vector.tensor_copy(sums_acc[0:1, g, :], sums_ps[0:1, :])
        sumsT_ps = pp_u.tile([P, 2 * NJ], F32, tag=f"u{g}", bufs=1, name=f"sumsT{g}")
        for hf in range(2):
            for j in range(NJ):
                nc.tensor.transpose(
                    sumsT_ps[:, hf * NJ + j : hf * NJ + j + 1],
                    sums_acc[0:1, g, hf * GW + j * P : hf * GW + (j + 1) * P],
                    ident[0:1, 0:1],
                )
        rg = r_sb[:, g * 2 * NJ : (g + 1) * 2 * NJ]
        nc.vector.reciprocal(rg, sumsT_ps)
        nc.vector.tensor_scalar_mul(
            r_sb[:, g * 2 * NJ + NJ : (g + 1) * 2 * NJ],
            r_sb[:, g * 2 * NJ + NJ : (g + 1) * 2 * NJ],
            lam,
        )
        post_ps = pp_u.tile([P, 2 * NJ, P], F32, tag=f"u{g}", bufs=1, name=f"post{g}")
        for j in range(NJ):
            nc.tensor.transpose(
                post_ps[:, j, :], u_sb[:, j * P : (j + 1) * P], ident
            )
            nc.tensor.transpose(
                post_ps[:, NJ + j, :], u_sb[:, GW + j * P : GW + (j + 1) * P], ident
            )
        attns = []
        for j in range(NJ):
            rcol = g * 2 * NJ
            idx = g * NJ + j
            t2 = small.tile([P, P], F32, tag="t2")
            nc.vector.scalar_tensor_tensor(
                t2,
                post_ps[:, NJ + j, :],
                r_sb[:, rcol + NJ + j : rcol + NJ + j + 1],
                rmsw_bc,
                op0=OP.mult,
                op1=OP.bypass,
            )
            attn_sb = attn_pool.tile([P, P], F32, tag="attn")
            nc.vector.scalar_tensor_tensor(
                attn_sb,
                post_ps[:, j, :],
                r_sb[:, rcol + j : rcol + j + 1],
                t2,
                op0=OP.mult,
                op1=OP.subtract,
            )
            sq_scr = small.tile([P, P], F32, tag="sqscr")
            # scale folds 1/(H*c^2) into the squares; the eps term is 5
            # orders below the signal and is dropped
            nc.scalar.activation(
                sq_scr,
                attn_sb,
                AF.Square,
                scale=math.sqrt(a_),
                accum_out=rmsall[:, idx : idx + 1],
            )
            attns.append(attn_sb)
        gs = slice(g * NJ, (g + 1) * NJ)
        roots = small.tile([P, NJ], F32, tag="roots")
        nc.scalar.activation(roots, rmsall[:, gs], AF.Sqrt)
        rrms = small.tile([P, NJ], F32, tag="rrms")
        nc.vector.reciprocal(rrms, roots)
        o_g = outp.tile([P, NJ, H], F32, tag=f"o{g}", bufs=1, name=f"o{g}")
        for j in range(NJ):
            nc.vector.scalar_tensor_tensor(
                o_g[:, j, :],
                attns[j],
                rrms[:, j : j + 1],
                rmsw_bc,
                op0=OP.mult,
                op1=OP.mult,
            )
        orows = out_d[g * GW : (g + 1) * GW, :]
        nc.sync.dma_start(out=orows.rearrange("(j p) h -> p j h", p=P), in_=o_g)

    for blk in range(NBLK):
        for g in range(NGROUPS):
            if blk == NBLK - 1 and g == 1:
                # group 0 just finished its final PV (pending) — flush and
                # emit its whole tail so it overlaps g1's last block
                flush_pending()
                group_tail(0)
            q0 = g * GW
            for ch in range(BCH):
                s_ps = pp_s.tile([P, 2 * GW], F32, tag="s", name="s_ps")
                k0 = blk  # slot
                nc.tensor.matmul(
                    s_ps[:, 0:GW],
                    kT_sb[0:H2, k0, ch * P : (ch + 1) * P],
                    qT_sb[0:H2, q0 : q0 + GW],
                )
                nc.tensor.matmul(
                    s_ps[:, GW : 2 * GW],
                    kT_sb[H2:H, k0, ch * P : (ch + 1) * P],
                    qT_sb[H2:H, q0 : q0 + GW],
                )
                e_sb = epool.tile([P, 2 * GW], F16, tag="e", name="e_sb")
                nc.scalar.activation(e_sb, s_ps, AF.Exp, scale=SCALE, bias=nbias)
                flush_pending()
                pending = (e_sb, g, blk, ch)
    flush_pending()
    group_tail(1)


def build(lam: float):
    from concourse._compat import axon_active

    nc = bacc.Bacc(
        "TRN2",
        target_bir_lowering=False,
        debug=not axon_active(),
        num_devices=NCORES,
    )
    with tile.TileContext(nc) as tc:
        with ExitStack() as ctx:
            _emit(ctx, tc, lam)
    nc.compile()
    return nc


def make_in_maps(x, Wq, Wk, Wv, rms_weight):
    bf = ml_dtypes.bfloat16
    x = np.asarray(x, dtype=np.float32)
    xT = np.ascontiguousarray(x.transpose(0, 2, 1)).astype(bf)  # [B, D, S]
    wqT = np.ascontiguousarray(np.asarray(Wq, np.float32).T).astype(bf)
    wkT = np.ascontiguousarray(np.asarray(Wk, np.float32).T).astype(bf)
    wvT = np.ascontiguousarray(np.asarray(Wv, np.float32).T).astype(bf)
    rw = np.ascontiguousarray(np.asarray(rms_weight, np.float32))
    in_maps = []
    for core in range(NCORES):
        b, qb = divmod(core, NCORES // B)
        xb = np.empty((D, 3, BLKW), bf)
        for t in range(3):
            rb = qb ^ t
            xb[:, t, :] = xT[b][:, rb * BLKW : (rb + 1) * BLKW]
        in_maps.append(
            {
                "xb": np.ascontiguousarray(xb),
                "wqT": wqT,
                "wkT": wkT,
                "wvT": wvT,
                "rmsw": rw,
            }
        )
    return in_maps


def kernel(x, Wq, Wk, Wv, lambda_q1, lambda_q2, lambda_k1, lambda_k2, rms_weight):
    lq1 = np.asarray(lambda_q1, np.float32)
    lq2 = np.asarray(lambda_q2, np.float32)
    lk1 = np.asarray(lambda_k1, np.float32)
    lk2 = np.asarray(lambda_k2, np.float32)
    lam = float(
        np.exp(np.dot(lq1, lk1)) - np.exp(np.dot(lq2, lk2)) + LAMBDA_INIT
    )
    nc = build(lam)
    in_maps = make_in_maps(x, Wq, Wk, Wv, rms_weight)
    res = bass_utils.run_bass_kernel_spmd(nc, in_maps, core_ids=list(range(NCORES)))
    out = np.empty((B, S, H), np.float32)
    for core in range(NCORES):
        b, qb = divmod(core, NCORES // B)
        out[b, qb * QSHARD : (qb + 1) * QSHARD] = res.results[core]["out"]
    return out


# revision 32
# speedup vs baseline: 1.5596x; 1.0004x over previous
"""DiffAttn (differential attention) Trainium2 Bass kernel, v2.

Self-contained: kernel(**inputs) takes the FULL unsharded inputs as numpy
arrays and returns the FULL output [2, 4096, 128] float32.

Sharding: 8 cores = (batch in {0,1}) x (query-block of 1024 rows).  Key
blocks are consumed in XOR-delta order (softmax over keys is permutation
invariant, so block order is irrelevant): the host stages each core's x
slices pre-permuted as [own, own^1, own^2], the core projects K/V for those
three blocks locally, and block own^3 is exchanged through a tiny PAIRWISE
AllGather ([[0,3],[1,2],[4,7],[5,6]]) whose output holds {own, own^3} in
rank order -- both halves are attended (own is NOT attended from the local
projection), which keeps the program identity-free SPMD.

Scores are computed TRANSPOSED ([sk, sq], keys on partitions) so exp(scores)
feeds the PV matmul directly as the moving operand (attention contracts over
sk).  Softmax row-sums are NOT computed with per-chunk ones-matmuls (that
costs as much PE as the PV itself): instead the DVE accumulates e chunk-wise
into an fp16 running sum tile and a single ones-matmul per group recovers
the sums at the end of the sweep.  exp is biased by -EXP_BIAS (cancels in
u/sums) so the fp16 accumulation cannot overflow.

The main loop is ACT(exp)-bound; emission is software-pipelined depth-1
(scores(i+1) issues before PV(i)) which keeps the ACT engine saturated.
"""

import math
import os
import sys
from contextlib import ExitStack

import numpy as np

for _p in ("/root/.axon_site/_ro/trn_rl_repo", "/opt/trn_rl_repo"):
    if os.path.isdir(_p) and _p not in sys.path:
        sys.path.append(_p)

import ml_dtypes  # noqa: E402

import concourse.bass as bass  # noqa: E402
import concourse.mybir as mybir  # noqa: E402
import concourse.tile as tile  # noqa: E402
from concourse import bacc, bass_utils  # noqa: E402
from concourse.masks import make_identity  # noqa: E402

B, S, D, H = 2, 4096, 2048, 128
H2 = H // 2  # 64
P = 128
NCORES = 8
QSHARD = 1024  # q rows per core
DCH = D // P  # 16 d-chunks
NGROUPS, GW = 2, 512  # q groups per core
NBLK, BLKW = 4, 1024  # attended key blocks per core
BCH = BLKW // P  # 8 key chunks per block
NJ = GW // P  # 4 q sub-blocks of 128 per group

LAMBDA_INIT = 0.8 - 0.6 * math.exp(-0.3 * 12)
RMS_EPS = float(np.finfo(np.float32).eps)
SCALE = 1.0 / math.sqrt(H2)
EXP_BIAS = 4.0  # exp(s*SCALE - EXP_BIAS); cancels in u/sums

F32 = mybir.dt.float32
BF16 = mybir.dt.bfloat16
F16 = mybir.dt.float16

AF = mybir.ActivationFunctionType
OP = mybir.AluOpType


def _emit(ctx: ExitStack, tc: "tile.TileContext", lam: float):
    nc = tc.nc

    # x slices for blocks [own, own^1, own^2] (host pre-permutes per core)
    xb = nc.dram_tensor("xb", (D, 3, BLKW), BF16, kind="ExternalInput").ap()
    part_d = nc.dram_tensor("part_d", (2, P, BLKW), F16).ap()
    pair_d = nc.dram_tensor("pair_d", (4, P, BLKW), F16).ap()
    wqT = nc.dram_tensor("wqT", (D, H), BF16, kind="ExternalInput").ap()
    wkT = nc.dram_tensor("wkT", (D, H), BF16, kind="ExternalInput").ap()
    wvT = nc.dram_tensor("wvT", (D, H), BF16, kind="ExternalInput").ap()
    rmsw = nc.dram_tensor("rmsw", (H,), F32, kind="ExternalInput").ap()
    out_d = nc.dram_tensor("out", (QSHARD, H), F32, kind="ExternalOutput").ap()

    # ---- constant / persistent SBUF tiles ----
    consts = ctx.enter_context(tc.tile_pool(name="consts", bufs=1))
    persist = ctx.enter_context(tc.tile_pool(name="persist", bufs=1))

    ident = consts.tile([P, P], F32)
    make_identity(nc, ident)
    ident16 = consts.tile([P, P], F16)
    make_identity(nc, ident16)
    ones_f16 = consts.tile([P, 1], F16)
    nc.vector.memset(ones_f16, 1.0)
    nbias = consts.tile([P, 1], F32)
    nc.vector.memset(nbias, -EXP_BIAS)
    rmsw_bc = consts.tile([P, H], F32)
    nc.sync.dma_start(
        out=rmsw_bc,
        in_=bass.AP(tensor=rmsw.tensor, offset=0, ap=[[0, P], [1, H]]),
    )
    wq_sb = consts.tile([P, DCH, H], BF16)
    wk_sb = consts.tile([P, DCH, H], BF16)
    wv_sb = consts.tile([P, DCH, H], BF16)

    qT_sb = persist.tile([P, QSHARD], F16)  # [h, sq]
    # attended key blocks: slot 0 = own^1, 1 = own^2, 2/3 = pair {own, own^3}
    kT_sb = persist.tile([P, NBLK, BLKW], F16)  # [h, blk, sk]
    v_sb = persist.tile([P, NBLK, BCH, P], F16)  # [sk%128, blk, chunk, h]
    kpart_sb = persist.tile([P, BLKW], F16)  # own k (collective input only)
    vpart_sb = persist.tile([P, BCH, P], F16)  # own v (collective input only)

    xpool = ctx.enter_context(tc.tile_pool(name="xstream", bufs=3))
    epool = ctx.enter_context(tc.tile_pool(name="epool", bufs=6))
    accp = ctx.enter_context(tc.tile_pool(name="accp", bufs=1))
    usb_pool = ctx.enter_context(tc.tile_pool(name="usb", bufs=2))
    small = ctx.enter_context(tc.tile_pool(name="small", bufs=8))
    outp = ctx.enter_context(tc.tile_pool(name="outp", bufs=4))
    attn_pool = ctx.enter_context(tc.tile_pool(name="attnp", bufs=2 * NGROUPS * NJ + 1))

    xb_r = xb.rearrange("(c p) t q -> p c t q", p=P)

    # x piece-wise uploads so the c-outer projections chase the DMA
    nc.sync.dma_start(out=wk_sb, in_=wkT.rearrange("(c p) h -> p c h", p=P))
    x_tiles = []
    for t in range(3):
        x_sb = xpool.tile([P, DCH, BLKW], BF16, tag="x", name=f"x{t}")
        npc = 16 if t == 0 else 8
        step = DCH // npc
        for pc in range(npc):
            nc.sync.dma_start(
                out=x_sb[:, pc * step : (pc + 1) * step, :],
                in_=xb_r[:, pc * step : (pc + 1) * step, t, :],
            )
            if t == 0 and pc == 0:
                nc.sync.dma_start(
                    out=wv_sb, in_=wvT.rearrange("(c p) h -> p c h", p=P)
                )
            if t == 0 and pc == 5:
                nc.sync.dma_start(
                    out=wq_sb, in_=wqT.rearrange("(c p) h -> p c h", p=P)
                )
        x_tiles.append(x_sb)

    vT_scr = persist.tile([P, BLKW], F16)  # scratch: vT before transposing

    def proj_kv(x_sb, kdst, vdst):
        """c-outer K + transposed-V projection (both chase the x pieces),
        then PE-transpose vT back to the natural [sk, h] layout."""
        ks = [
            pp_proj.tile([P, GW], F32, tag=f"kacc{i}", bufs=1, name=f"kacc{i}")
            for i in range(2)
        ]
        vs = [
            pp_proj.tile([P, GW], F32, tag=f"vacc{i}", bufs=1, name=f"vacc{i}")
            for i in range(2)
        ]
        for c in range(DCH):
            for sl in range(2):
                xs = x_sb[:, c, sl * GW : (sl + 1) * GW]
                nc.tensor.matmul(
                    ks[sl], wk_sb[:, c, :], xs, start=(c == 0), stop=(c == DCH - 1)
                )
                nc.tensor.matmul(
                    vs[sl], wv_sb[:, c, :], xs, start=(c == 0), stop=(c == DCH - 1)
                )
        for sl in range(2):
            nc.scalar.copy(kdst[:, sl * GW : (sl + 1) * GW], ks[sl])
            nc.scalar.copy(vT_scr[:, sl * GW : (sl + 1) * GW], vs[sl])
        for j in range(BCH):
            tp = pp_proj.tile([P, P], F16, tag="tp", bufs=2)
            nc.tensor.transpose(tp, vT_scr[:, j * P : (j + 1) * P], ident16)
            nc.vector.tensor_copy(vdst[:, j, :], tp)

    with tc.tile_pool(name="pp_proj", space="PSUM", bufs=1) as pp_proj:
        # PE p-state warm-up while the first x pieces are in flight: the
        # Tensor engine reaches full clock after ~3us of continuous work
        for w in range(48):
            wtp = pp_proj.tile([P, P], F16, tag="tp", bufs=2, name="warm")
            nc.tensor.transpose(wtp, ident16, ident16)
        # own block: project K/V, ship through the pairwise collective
        proj_kv(x_tiles[0], kpart_sb, vpart_sb)
        nc.sync.dma_start(out=part_d[0], in_=kpart_sb)
        nc.sync.dma_start(out=part_d[1], in_=vpart_sb.rearrange("p j h -> p (j h)"))
        nc.gpsimd.collective_compute(
            "AllGather",
            OP.bypass,
            replica_groups=[[0, 3], [1, 2], [4, 7], [5, 6]],
            ins=[part_d.opt()],
            outs=[pair_d.opt()],
        )
        # q projection + d1/d2 K/V projections overlap the collective flight
        for sl in range(2):
            qacc = pp_proj.tile([P, GW], F32, tag="kacc0", bufs=1)
            for c in range(DCH):
                nc.tensor.matmul(
                    qacc,
                    wq_sb[:, c, :],
                    x_tiles[0][:, c, sl * GW : (sl + 1) * GW],
                    start=(c == 0),
                    stop=(c == DCH - 1),
                )
            nc.scalar.copy(qT_sb[:, sl * GW : (sl + 1) * GW], qacc)
        for t in (1, 2):
            proj_kv(x_tiles[t], kT_sb[:, t - 1, :], v_sb[:, t - 1, :, :])

    # unpack the gathered pair {own, own^3} into slots 2, 3
    for r in range(2):
        nc.sync.dma_start(out=kT_sb[:, 2 + r, :], in_=pair_d[2 * r])
        nc.sync.dma_start(
            out=v_sb[:, 2 + r, :, :],
            in_=pair_d[2 * r + 1].rearrange("p (j h) -> p j h", j=BCH),
        )

    # ---- attention sweep: 4 blocks x 2 groups x 8 chunks, pipelined ----
    pp_s = ctx.enter_context(tc.tile_pool(name="pp_s", space="PSUM", bufs=2))
    pp_u = ctx.enter_context(tc.tile_pool(name="pp_u", space="PSUM", bufs=1))

    # per-group u accumulators; (g, hf) accumulation groups in disjoint banks
    u_ps = [
        pp_u.tile([P, 2, GW], F32, tag=f"u{g}", bufs=1, name=f"u{g}")
        for g in range(NGROUPS)
    ]
    acc = [accp.tile([P, 2 * GW], F16, tag=f"acc{g}", name=f"acc{g}") for g in range(2)]

    pending = None  # (e_sb, g, blk, ch) awaiting PV + DVE accumulation

    def flush_pending():
        nonlocal pending
        if pending is None:
            return
        e_sb, g, blk, ch = pending
        first = blk == 0 and ch == 0
        last = blk == NBLK - 1 and ch == BCH - 1
        for hf in range(2):
            nc.tensor.matmul(
                u_ps[g][:, hf, :],
                v_sb[:, blk, ch, :],
                e_sb[:, hf * GW : (hf + 1) * GW],
                start=first,
                stop=last,
            )
        if first:
            nc.vector.tensor_copy(acc[g], e_sb)
        else:
            nc.vector.tensor_tensor(acc[g], acc[g], e_sb, op=OP.add)
        pending = None

    c_ = 1.0 - LAMBDA_INIT
    a_ = 1.0 / (H * c_ * c_)
    b_ = RMS_EPS / (c_ * c_)
    sums_acc = small.tile([1, NGROUPS, 2 * GW], F32, tag="sums_acc", bufs=1)
    r_sb = small.tile([P, 2 * 2 * NJ], F32, tag="r", bufs=1)
    rmsall = small.tile([P, NGROUPS * NJ], F32, tag="rmsall", bufs=1)
    u_sbs = {}

    def group_tail(g):
        """u -> SBUF, sums, normalization factors, combine, RMS, store —
        emitted right after group g's final PV so g0's tail overlaps g1's
        last block of attention."""
        u_sb = usb_pool.tile([P, 2 * GW], F32, tag="usb")
        nc.vector.tensor_copy(u_sb, u_ps[g][:, :, :])
        u_sbs[g] = u_sb
        sums_ps = pp_u.tile([P, 2 * GW], F32, tag=f"u{g}", bufs=1, name=f"sums{g}")
        for hf in range(2):
            sl = slice(hf * GW, (hf + 1) * GW)
            nc.tensor.matmul(sums_ps[0:1, sl], ones_f16, acc[g][:, sl])
        nc.vector.tensor_copy(sums_acc[0:1, g, :], sums_ps[0:1, :])
        sumsT_ps = pp_u.tile([P, 2 * NJ], F32, tag=f"u{g}", bufs=1, name=f"sumsT{g}")
        for hf in range(2):
            for j in range(NJ):
                nc.tensor.transpose(
                    sumsT_ps[:, hf * NJ + j : hf * NJ + j + 1],
                    sums_acc[0:1, g, hf * GW + j * P : hf * GW + (j + 1) * P],
                    ident[0:1, 0:1],
                )
        rg = r_sb[:, g * 2 * NJ : (g + 1) * 2 * NJ]
        nc.vector.reciprocal(rg, sumsT_ps)
        nc.vector.tensor_scalar_mul(
            r_sb[:, g * 2 * NJ + NJ : (g + 1) * 2 * NJ],
            r_sb[:, g * 2 * NJ + NJ : (g + 1) * 2 * NJ],
            lam,
        )
        post_ps = pp_u.tile([P, 2 * NJ, P], F32, tag=f"u{g}", bufs=1, name=f"post{g}")
        for j in range(NJ):
            nc.tensor.transpose(
                post_ps[:, j, :], u_sb[:, j * P : (j + 1) * P], ident
            )
            nc.tensor.transpose(
                post_ps[:, NJ + j, :], u_sb[:, GW + j * P : GW + (j + 1) * P], ident
            )
        attns = []
        for j in range(NJ):
            rcol = g * 2 * NJ
            idx = g * NJ + j
            t2 = small.tile([P, P], F32, tag="t2")
            nc.vector.scalar_tensor_tensor(
                t2,
                post_ps[:, NJ + j, :],
                r_sb[:, rcol + NJ + j : rcol + NJ + j + 1],
                rmsw_bc,
                op0=OP.mult,
                op1=OP.bypass,
            )
            attn_sb = attn_pool.tile([P, P], F32, tag="attn")
            nc.vector.scalar_tensor_tensor(
                attn_sb,
                post_ps[:, j, :],
                r_sb[:, rcol + j : rcol + j + 1],
                t2,
                op0=OP.mult,
                op1=OP.subtract,
            )
            sq_scr = small.tile([P, P], F32, tag="sqscr")
            # scale folds 1/(H*c^2) into the squares; the eps term is 5
            # orders below the signal and is dropped
            nc.scalar.activation(
                sq_scr,
                attn_sb,
                AF.Square,
                scale=math.sqrt(a_),
                accum_out=rmsall[:, idx : idx + 1],
            )
            attns.append(attn_sb)
        gs = slice(g * NJ, (g + 1) * NJ)
        roots = small.tile([P, NJ], F32, tag="roots")
        nc.scalar.activation(roots, rmsall[:, gs], AF.Sqrt)
        rrms = small.tile([P, NJ], F32, tag="rrms")
        nc.vector.reciprocal(rrms, roots)
        o_g = outp.tile([P, NJ, H], F32, tag=f"o{g}", bufs=1, name=f"o{g}")
        for j in range(NJ):
            nc.vector.scalar_tensor_tensor(
                o_g[:, j, :],
                attns[j],
                rrms[:, j : j + 1],
                rmsw_bc,
                op0=OP.mult,
                op1=OP.mult,
            )
        orows = out_d[g * GW : (g + 1) * GW, :]
        nc.sync.dma_start(out=orows.rearrange("(j p) h -> p j h", p=P), in_=o_g)

    for blk in range(NBLK):
        for g in range(NGROUPS):
            if blk == NBLK - 1 and g == 1:
                # group 0 just finished its final PV (pending) — flush and
                # emit its whole tail so it overlaps g1's last block
                flush_pending()
                group_tail(0)
            q0 = g * GW
            for ch in range(BCH):
                s_ps = pp_s.tile([P, 2 * GW], F32, tag="s", name="s_ps")
                k0 = blk  # slot
                nc.tensor.matmul(
                    s_ps[:, 0:GW],
                    kT_sb[0:H2, k0, ch * P : (ch + 1) * P],
                    qT_sb[0:H2, q0 : q0 + GW],
                )
                nc.tensor.matmul(
                    s_ps[:, GW : 2 * GW],
                    kT_sb[H2:H, k0, ch * P : (ch + 1) * P],
                    qT_sb[H2:H, q0 : q0 + GW],
                )
                e_sb = epool.tile([P, 2 * GW], F16, tag="e", name="e_sb")
                nc.scalar.activation(e_sb, s_ps, AF.Exp, scale=SCALE, bias=nbias)
                flush_pending()
                pending = (e_sb, g, blk, ch)
    flush_pending()
    group_tail(1)


def build(lam: float):
    from concourse._compat import axon_active

    nc = bacc.Bacc(
        "TRN2",
        target_bir_lowering=False,
        debug=not axon_active(),
        num_devices=NCORES,
    )
    with tile.TileContext(nc) as tc:
        with ExitStack() as ctx:
            _emit(ctx, tc, lam)
    nc.compile()
    return nc


def make_in_maps(x, Wq, Wk, Wv, rms_weight):
    bf = ml_dtypes.bfloat16
    x = np.asarray(x, dtype=np.float32)
    xT = np.ascontiguousarray(x.transpose(0, 2, 1)).astype(bf)  # [B, D, S]
    wqT = np.ascontiguousarray(np.asarray(Wq, np.float32).T).astype(bf)
    wkT = np.ascontiguousarray(np.asarray(Wk, np.float32).T).astype(bf)
    wvT = np.ascontiguousarray(np.asarray(Wv, np.float32).T).astype(bf)
    rw = np.ascontiguousarray(np.asarray(rms_weight, np.float32))
    in_maps = []
    for core in range(NCORES):
        b, qb = divmod(core, NCORES // B)
        xb = np.empty((D, 3, BLKW), bf)
        for t in range(3):
            rb = qb ^ t
            xb[:, t, :] = xT[b][:, rb * BLKW : (rb + 1) * BLKW]
        in_maps.append(
            {
                "xb": np.ascontiguousarray(xb),
                "wqT": wqT,
                "wkT": wkT,
                "wvT": wvT,
                "rmsw": rw,
            }
        )
    return in_maps


def kernel(x, Wq, Wk, Wv, lambda_q1, lambda_q2, lambda_k1, lambda_k2, rms_weight):
    lq1 = np.asarray(lambda_q1, np.float32)
    lq2 = np.asarray(lambda_q2, np.float32)
    lk1 = np.asarray(lambda_k1, np.float32)
    lk2 = np.asarray(lambda_k2, np.float32)
    lam = float(
        np.exp(np.dot(lq1, lk1)) - np.exp(np.dot(lq2, lk2)) + LAMBDA_INIT
    )
    nc = build(lam)
    in_maps = make_in_maps(x, Wq, Wk, Wv, rms_weight)
    res = bass_utils.run_bass_kernel_spmd(nc, in_maps, core_ids=list(range(NCORES)))
    out = np.empty((B, S, H), np.float32)
    for core in range(NCORES):
        b, qb = divmod(core, NCORES // B)
        out[b, qb * QSHARD : (qb + 1) * QSHARD] = res.results[core]["out"]
    return out


# revision 37
# speedup vs baseline: 1.5711x; 1.0074x over previous
"""DiffAttn (differential attention) Trainium2 Bass kernel, v2.

Self-contained: kernel(**inputs) takes the FULL unsharded inputs as numpy
arrays and returns the FULL output [2, 4096, 128] float32.

Sharding: 8 cores = (batch in {0,1}) x (query-block of 1024 rows).  Key
blocks are consumed in XOR-delta order (softmax over keys is permutation
invariant, so block order is irrelevant): the host stages each core's x
slices pre-permuted as [own, own^1, own^2], the core projects K/V for those
three blocks locally, and block own^3 is exchanged through a tiny PAIRWISE
AllGather ([[0,3],[1,2],[4,7],[5,6]]) whose output holds {own, own^3} in
rank order -- both halves are attended (own is NOT attended from the local
projection), which keeps the program identity-free SPMD.

Scores are computed TRANSPOSED ([sk, sq], keys on partitions) so exp(scores)
feeds the PV matmul directly as the moving operand (attention contracts over
sk).  Softmax row-sums are NOT computed with per-chunk ones-matmuls (that
costs as much PE as the PV itself): instead the DVE accumulates e chunk-wise
into an fp16 running sum tile and a single ones-matmul per group recovers
the sums at the end of the sweep.  exp is biased by -EXP_BIAS (cancels in
u/sums) so the fp16 accumulation cannot overflow.

The main loop is ACT(exp)-bound; emission is software-pipelined depth-1
(scores(i+1) issues before PV(i)) which keeps the ACT engine saturated.
"""

import math
import os
import sys
from contextlib import ExitStack

import numpy as np

for _p in ("/root/.axon_site/_ro/trn_rl_repo", "/opt/trn_rl_repo"):
    if os.path.isdir(_p) and _p not in sys.path:
        sys.path.append(_p)

import ml_dtypes  # noqa: E402

import concourse.bass as bass  # noqa: E402
import concourse.mybir as mybir  # noqa: E402
import concourse.tile as tile  # noqa: E402
from concourse import bacc, bass_utils  # noqa: E402
from concourse.masks import make_identity  # noqa: E402

B, S, D, H = 2, 4096, 2048, 128
H2 = H // 2  # 64
P = 128
NCORES = 8
QSHARD = 1024  # q rows per core
DCH = D // P  # 16 d-chunks
NGROUPS, GW = 2, 512  # q groups per core
NBLK, BLKW = 4, 1024  # attended key blocks per core
BCH = BLKW // P  # 8 key chunks per block
NJ = GW // P  # 4 q sub-blocks of 128 per group

LAMBDA_INIT = 0.8 - 0.6 * math.exp(-0.3 * 12)
RMS_EPS = float(np.finfo(np.float32).eps)
SCALE = 1.0 / math.sqrt(H2)
EXP_BIAS = 4.0  # exp(s*SCALE - EXP_BIAS); cancels in u/sums

F32 = mybir.dt.float32
BF16 = mybir.dt.bfloat16
F16 = mybir.dt.float16

AF = mybir.ActivationFunctionType
OP = mybir.AluOpType


def _emit(ctx: ExitStack, tc: "tile.TileContext", lam: float):
    nc = tc.nc

    # x slices for blocks [own, own^1, own^2] (host pre-permutes per core)
    xb = nc.dram_tensor("xb", (D, 3, BLKW), BF16, kind="ExternalInput").ap()
    part_d = nc.dram_tensor("part_d", (2, P, BLKW), F16).ap()
    pair_d = nc.dram_tensor("pair_d", (4, P, BLKW), F16).ap()
    wqT = nc.dram_tensor("wqT", (D, H), BF16, kind="ExternalInput").ap()
    wkT = nc.dram_tensor("wkT", (D, H), BF16, kind="ExternalInput").ap()
    wvT = nc.dram_tensor("wvT", (D, H), BF16, kind="ExternalInput").ap()
    rmsw = nc.dram_tensor("rmsw", (H,), F32, kind="ExternalInput").ap()
    out_d = nc.dram_tensor("out", (QSHARD, H), F32, kind="ExternalOutput").ap()

    # ---- constant / persistent SBUF tiles ----
    consts = ctx.enter_context(tc.tile_pool(name="consts", bufs=1))
    persist = ctx.enter_context(tc.tile_pool(name="persist", bufs=1))

    ident = consts.tile([P, P], F32)
    make_identity(nc, ident)
    ident16 = consts.tile([P, P], F16)
    make_identity(nc, ident16)
    ones_f16 = consts.tile([P, 1], F16)
    nc.vector.memset(ones_f16, 1.0)
    nbias = consts.tile([P, 1], F32)
    nc.vector.memset(nbias, -EXP_BIAS)
    rmsw_bc = consts.tile([P, H], F32)
    nc.sync.dma_start(
        out=rmsw_bc,
        in_=bass.AP(tensor=rmsw.tensor, offset=0, ap=[[0, P], [1, H]]),
    )
    wq_sb = consts.tile([P, DCH, H], BF16)
    wk_sb = consts.tile([P, DCH, H], BF16)
    wv_sb = consts.tile([P, DCH, H], BF16)

    qT_sb = persist.tile([P, QSHARD], F16)  # [h, sq]
    # attended key blocks: slot 0 = own^1, 1 = own^2, 2/3 = pair {own, own^3}
    kT_sb = persist.tile([P, NBLK, BLKW], F16)  # [h, blk, sk]
    v_sb = persist.tile([P, NBLK, BCH, P], F16)  # [sk%128, blk, chunk, h]
    kpart_sb = persist.tile([P, BLKW], F16)  # own k (collective input only)
    vpart_sb = persist.tile([P, BCH, P], F16)  # own v (collective input only)

    xpool = ctx.enter_context(tc.tile_pool(name="xstream", bufs=3))
    epool = ctx.enter_context(tc.tile_pool(name="epool", bufs=6))
    accp = ctx.enter_context(tc.tile_pool(name="accp", bufs=1))
    usb_pool = ctx.enter_context(tc.tile_pool(name="usb", bufs=2))
    small = ctx.enter_context(tc.tile_pool(name="small", bufs=8))
    outp = ctx.enter_context(tc.tile_pool(name="outp", bufs=4))
    attn_pool = ctx.enter_context(tc.tile_pool(name="attnp", bufs=2 * NGROUPS * NJ + 1))

    xb_r = xb.rearrange("(c p) t q -> p c t q", p=P)

    # x0 uploads first; x1/x2 are emitted AFTER the part DMAs + collective
    # so the pairwise exchange dispatches at ~18us instead of queueing
    # behind 23us of x traffic
    nc.sync.dma_start(out=wk_sb, in_=wkT.rearrange("(c p) h -> p c h", p=P))
    x_tiles = [
        xpool.tile([P, DCH, BLKW], BF16, tag="x", name=f"x{t}") for t in range(3)
    ]

    def upload_x(t, npc):
        step = DCH // npc
        for pc in range(npc):
            nc.sync.dma_start(
                out=x_tiles[t][:, pc * step : (pc + 1) * step, :],
                in_=xb_r[:, pc * step : (pc + 1) * step, t, :],
            )
            if t == 0 and pc == 0:
                nc.sync.dma_start(
                    out=wv_sb, in_=wvT.rearrange("(c p) h -> p c h", p=P)
                )
            if t == 0 and pc == 5:
                nc.sync.dma_start(
                    out=wq_sb, in_=wqT.rearrange("(c p) h -> p c h", p=P)
                )

    upload_x(0, 16)

    vT_scr = persist.tile([P, BLKW], F16)  # scratch: vT before transposing

    def proj_par(pool, x_sb, kdst, vdst):
        """4-bank parallel c-outer K + vT projection chasing the x pieces."""
        ks = [
            pool.tile([P, GW], F32, tag=f"kacc{i}", bufs=1, name=f"kacc{i}")
            for i in range(2)
        ]
        vs = [
            pool.tile([P, GW], F32, tag=f"vacc{i}", bufs=1, name=f"vacc{i}")
            for i in range(2)
        ]
        for c in range(DCH):
            for sl in range(2):
                xs = x_sb[:, c, sl * GW : (sl + 1) * GW]
                nc.tensor.matmul(
                    ks[sl], wk_sb[:, c, :], xs, start=(c == 0), stop=(c == DCH - 1)
                )
                nc.tensor.matmul(
                    vs[sl], wv_sb[:, c, :], xs, start=(c == 0), stop=(c == DCH - 1)
                )
        for sl in range(2):
            nc.scalar.copy(kdst[:, sl * GW : (sl + 1) * GW], ks[sl])
            nc.scalar.copy(vT_scr[:, sl * GW : (sl + 1) * GW], vs[sl])
        for j in range(BCH):
            tp = pool.tile([P, P], F16, tag=f"vacc{j % 2}", bufs=1, name=f"tp{j % 2}")
            nc.tensor.transpose(tp, vT_scr[:, j * P : (j + 1) * P], ident16)
            nc.vector.tensor_copy(vdst[:, j, :], tp)

    def proj_quanta(pool, x_sb, kdst, vdst):
        """2-bank serialized projection as emission quanta, interleaved with
        the group-0 attention sweep."""
        q = []
        ks = [
            pool.tile([P, GW], F32, tag=f"pk{i}", bufs=1, name=f"pk{i}")
            for i in range(2)
        ]

        def kmm(c, sl):
            nc.tensor.matmul(
                ks[sl],
                wk_sb[:, c, :],
                x_sb[:, c, sl * GW : (sl + 1) * GW],
                start=(c == 0),
                stop=(c == DCH - 1),
            )

        for c in range(DCH):
            q.append(lambda c=c: (kmm(c, 0), kmm(c, 1)))
        q.append(
            lambda: [
                nc.scalar.copy(kdst[:, sl * GW : (sl + 1) * GW], ks[sl])
                for sl in range(2)
            ]
        )
        vst = {}

        def valloc():
            vst[0] = pool.tile([P, GW], F32, tag="pk0", bufs=1, name="pv0")
            vst[1] = pool.tile([P, GW], F32, tag="pk1", bufs=1, name="pv1")

        def vmm(c, sl):
            nc.tensor.matmul(
                vst[sl],
                wv_sb[:, c, :],
                x_sb[:, c, sl * GW : (sl + 1) * GW],
                start=(c == 0),
                stop=(c == DCH - 1),
            )

        q.append(valloc)
        for c in range(DCH):
            q.append(lambda c=c: (vmm(c, 0), vmm(c, 1)))
        q.append(
            lambda: [
                nc.scalar.copy(vT_scr[:, sl * GW : (sl + 1) * GW], vst[sl])
                for sl in range(2)
            ]
        )

        def tpj(j):
            tp = pool.tile([P, P], F16, tag=f"pk{j % 2}", bufs=1, name=f"ptp{j % 2}")
            nc.tensor.transpose(tp, vT_scr[:, j * P : (j + 1) * P], ident16)
            nc.vector.tensor_copy(vdst[:, j, :], tp)

        for j in range(BCH):
            q.append(lambda j=j: tpj(j))
        return q

    # ---- phase A: own + d1 projections (4-bank pool, closed after) ----
    with tc.tile_pool(name="pp_proj4", space="PSUM", bufs=1) as pp_proj4:
        for w in range(48):
            wtp = pp_proj4.tile(
                [P, P], F16, tag=f"vacc{w % 2}", bufs=1, name=f"tp{w % 2}"
            )
            nc.tensor.transpose(wtp, ident16, ident16)
        proj_par(pp_proj4, x_tiles[0], kpart_sb, vpart_sb)
        nc.sync.dma_start(out=part_d[0], in_=kpart_sb)
        nc.sync.dma_start(out=part_d[1], in_=vpart_sb.rearrange("p j h -> p (j h)"))
        nc.gpsimd.collective_compute(
            "AllGather",
            OP.bypass,
            replica_groups=[[0, 3], [1, 2], [4, 7], [5, 6]],
            ins=[part_d.opt()],
            outs=[pair_d.opt()],
        )
        upload_x(1, 8)
        upload_x(2, 8)
        # group-0 half of q only; g1's half projects inside the g0 sweep
        qacc = pp_proj4.tile([P, GW], F32, tag="kacc0", bufs=1, name="qacc")
        for c in range(DCH):
            nc.tensor.matmul(
                qacc,
                wq_sb[:, c, :],
                x_tiles[0][:, c, 0:GW],
                start=(c == 0),
                stop=(c == DCH - 1),
            )
        nc.scalar.copy(qT_sb[:, 0:GW], qacc)
        proj_par(pp_proj4, x_tiles[1], kT_sb[:, 0, :], v_sb[:, 0, :, :])

    # unpack the gathered pair {own, own^3} into slots 2, 3
    for r in range(2):
        nc.sync.dma_start(out=kT_sb[:, 2 + r, :], in_=pair_d[2 * r])
        nc.sync.dma_start(
            out=v_sb[:, 2 + r, :, :],
            in_=pair_d[2 * r + 1].rearrange("p (j h) -> p j h", j=BCH),
        )

    # ---- g-major sweeps: only one u accumulator live at a time, which
    # frees 2 PSUM banks during g0's sweep for the in-loop d2 projection ----
    pp_s = ctx.enter_context(tc.tile_pool(name="pp_s", space="PSUM", bufs=2))
    pp_u0 = ctx.enter_context(tc.tile_pool(name="pp_u0", space="PSUM", bufs=1))
    pools_u = {0: pp_u0}
    u_ps = {0: pp_u0.tile([P, 2, GW], F32, tag="u0", bufs=1, name="u0")}
    acc = [accp.tile([P, 2 * GW], F16, tag=f"acc{g}", name=f"acc{g}") for g in range(2)]

    pending = None  # (e_sb, g, slot, ch, first, last)

    def flush_pending():
        nonlocal pending
        if pending is None:
            return
        e_sb, g, slot, ch, first, last = pending
        for hf in range(2):
            nc.tensor.matmul(
                u_ps[g][:, hf, :],
                v_sb[:, slot, ch, :],
                e_sb[:, hf * GW : (hf + 1) * GW],
                start=first,
                stop=last,
            )
        if first:
            nc.vector.tensor_copy(acc[g], e_sb)
        else:
            nc.vector.tensor_tensor(acc[g], acc[g], e_sb, op=OP.add)
        pending = None

    def attend(g, slot, ch, first, last):
        nonlocal pending
        q0 = g * GW
        s_ps = pp_s.tile([P, 2 * GW], F32, tag="s", name="s_ps")
        nc.tensor.matmul(
            s_ps[:, 0:GW],
            kT_sb[0:H2, slot, ch * P : (ch + 1) * P],
            qT_sb[0:H2, q0 : q0 + GW],
        )
        nc.tensor.matmul(
            s_ps[:, GW : 2 * GW],
            kT_sb[H2:H, slot, ch * P : (ch + 1) * P],
            qT_sb[H2:H, q0 : q0 + GW],
        )
        e_sb = epool.tile([P, 2 * GW], F16, tag="e", name="e_sb")
        nc.scalar.activation(e_sb, s_ps, AF.Exp, scale=SCALE, bias=nbias)
        flush_pending()
        pending = (e_sb, g, slot, ch, first, last)

    c_ = 1.0 - LAMBDA_INIT
    a_ = 1.0 / (H * c_ * c_)
    b_ = RMS_EPS / (c_ * c_)
    sums_acc = small.tile([1, NGROUPS, 2 * GW], F32, tag="sums_acc", bufs=1)
    r_sb = small.tile([P, 2 * 2 * NJ], F32, tag="r", bufs=1)
    rmsall = small.tile([P, NGROUPS * NJ], F32, tag="rmsall", bufs=1)
    u_sbs = {}
    attns_all = {}

    def group_tail(g):
        """u -> SBUF, sums, normalization factors, combine, RMS, store —
        emitted right after group g's final PV so g0's tail overlaps g1's
        last block of attention."""
        u_sb = usb_pool.tile([P, 2 * GW], F32, tag="usb")
        nc.vector.tensor_copy(u_sb, u_ps[g][:, :, :])
        u_sbs[g] = u_sb
        sums_ps = pools_u[g].tile([P, 2 * GW], F32, tag=f"u{g}", bufs=1, name=f"sums{g}")
        for hf in range(2):
            sl = slice(hf * GW, (hf + 1) * GW)
            nc.tensor.matmul(sums_ps[0:1, sl], ones_f16, acc[g][:, sl])
        nc.vector.tensor_copy(sums_acc[0:1, g, :], sums_ps[0:1, :])
        sumsT_ps = pools_u[g].tile([P, 2 * NJ], F32, tag=f"u{g}", bufs=1, name=f"sumsT{g}")
        for hf in range(2):
            for j in range(NJ):
                nc.tensor.transpose(
                    sumsT_ps[:, hf * NJ + j : hf * NJ + j + 1],
                    sums_acc[0:1, g, hf * GW + j * P : hf * GW + (j + 1) * P],
                    ident[0:1, 0:1],
                )
        rg = r_sb[:, g * 2 * NJ : (g + 1) * 2 * NJ]
        nc.vector.reciprocal(rg, sumsT_ps)
        nc.vector.tensor_scalar_mul(
            r_sb[:, g * 2 * NJ + NJ : (g + 1) * 2 * NJ],
            r_sb[:, g * 2 * NJ + NJ : (g + 1) * 2 * NJ],
            lam,
        )
        post_ps = pools_u[g].tile([P, 2 * NJ, P], F32, tag=f"u{g}", bufs=1, name=f"post{g}")
        for j in range(NJ):
            nc.tensor.transpose(
                post_ps[:, j, :], u_sb[:, j * P : (j + 1) * P], ident
            )
            nc.tensor.transpose(
                post_ps[:, NJ + j, :], u_sb[:, GW + j * P : GW + (j + 1) * P], ident
            )
        attns = []
        for j in range(NJ):
            rcol = g * 2 * NJ
            idx = g * NJ + j
            t2 = small.tile([P, P], F32, tag="t2")
            nc.vector.scalar_tensor_tensor(
                t2,
                post_ps[:, NJ + j, :],
                r_sb[:, rcol + NJ + j : rcol + NJ + j + 1],
                rmsw_bc,
                op0=OP.mult,
                op1=OP.bypass,
            )
            attn_sb = attn_pool.tile([P, P], F32, tag="attn")
            nc.vector.scalar_tensor_tensor(
                attn_sb,
                post_ps[:, j, :],
                r_sb[:, rcol + j : rcol + j + 1],
                t2,
                op0=OP.mult,
                op1=OP.subtract,
            )
            sq_scr = small.tile([P, P], F32, tag="sqscr")
            # scale folds 1/(H*c^2) into the squares; the eps term is 5
            # orders below the signal and is dropped
            nc.scalar.activation(
                sq_scr,
                attn_sb,
                AF.Square,
                scale=math.sqrt(a_),
                accum_out=rmsall[:, idx : idx + 1],
            )
            attns.append(attn_sb)
        attns_all[g] = attns

    def final_tail():
        # ONE Sqrt for both groups: avoids a second Exp<->Sqrt ACT-table
        # switch inside the ACT-bound sweep
        roots = small.tile([P, NGROUPS * NJ], F32, tag="roots", bufs=1)
        nc.scalar.activation(roots, rmsall, AF.Sqrt)
        rrms = small.tile([P, NGROUPS * NJ], F32, tag="rrms", bufs=1)
        nc.vector.reciprocal(rrms, roots)
        o_all = outp.tile([P, NGROUPS * NJ, H], F32, tag="o", bufs=1, name="o")
        for g in range(NGROUPS):
            for j in range(NJ):
                idx = g * NJ + j
                nc.vector.scalar_tensor_tensor(
                    o_all[:, idx, :],
                    attns_all[g][j],
                    rrms[:, idx : idx + 1],
                    rmsw_bc,
                    op0=OP.mult,
                    op1=OP.mult,
                )
        nc.sync.dma_start(
            out=out_d.rearrange("(j p) h -> p j h", p=P), in_=o_all
        )

    # ---- phase B: g0 sweeps all 4 blocks; d2 + q(g1) project in the
    # PE slack using the 2 banks that u1 does not occupy yet ----
    with tc.tile_pool(name="pp_proj2", space="PSUM", bufs=1) as pp_proj2:
        quanta = proj_quanta(pp_proj2, x_tiles[2], kT_sb[:, 1, :], v_sb[:, 1, :, :])

        def qg1_quanta():
            q = []
            st = {}

            def alloc():
                st[0] = pp_proj2.tile([P, GW], F32, tag="pk0", bufs=1, name="qg1")

            q.append(alloc)

            def qmm(c):
                nc.tensor.matmul(
                    st[0],
                    wq_sb[:, c, :],
                    x_tiles[0][:, c, GW : 2 * GW],
                    start=(c == 0),
                    stop=(c == DCH - 1),
                )

            for c in range(DCH):
                q.append(lambda c=c: qmm(c))
            q.append(lambda: nc.scalar.copy(qT_sb[:, GW : 2 * GW], st[0]))
            return q

        quanta += qg1_quanta()
        qi = 0
        seq0 = [(s, c) for s in (0, 1, 2, 3) for c in range(BCH)]
        for i, (slot, ch) in enumerate(seq0):
            attend(0, slot, ch, first=(i == 0), last=(i == len(seq0) - 1))
            # d2's projection must land before its first attend (i == 8):
            # drain most quanta across the first 8 chunks, the rest after
            budget = 8 if i < 7 else len(quanta)
            took = 0
            while qi < len(quanta) and took < budget:
                quanta[qi]()
                qi += 1
                took += 1
        flush_pending()

    # ---- phase C: u1 takes the banks proj2 released; g0's tail overlaps ----
    pp_u1 = ctx.enter_context(tc.tile_pool(name="pp_u1", space="PSUM", bufs=1))
    pools_u[1] = pp_u1
    u_ps[1] = pp_u1.tile([P, 2, GW], F32, tag="u1", bufs=1, name="u1")
    seq1 = [(s, c) for s in range(NBLK) for c in range(BCH)]
    for i, (slot, ch) in enumerate(seq1):
        attend(1, slot, ch, first=(i == 0), last=(i == len(seq1) - 1))
        if i == 1:
            group_tail(0)
    flush_pending()
    group_tail(1)
    final_tail()


def build(lam: float):
    from concourse._compat import axon_active

    nc = bacc.Bacc(
        "TRN2",
        target_bir_lowering=False,
        debug=not axon_active(),
        num_devices=NCORES,
    )
    with tile.TileContext(nc) as tc:
        with ExitStack() as ctx:
            _emit(ctx, tc, lam)
    nc.compile()
    return nc


def make_in_maps(x, Wq, Wk, Wv, rms_weight):
    bf = ml_dtypes.bfloat16
    x = np.asarray(x, dtype=np.float32)
    xT = np.ascontiguousarray(x.transpose(0, 2, 1)).astype(bf)  # [B, D, S]
    wqT = np.ascontiguousarray(np.asarray(Wq, np.float32).T).astype(bf)
    wkT = np.ascontiguousarray(np.asarray(Wk, np.float32).T).astype(bf)
    wvT = np.ascontiguousarray(np.asarray(Wv, np.float32).T).astype(bf)
    rw = np.ascontiguousarray(np.asarray(rms_weight, np.float32))
    in_maps = []
    for core in range(NCORES):
        b, qb = divmod(core, NCORES // B)
        xb = np.empty((D, 3, BLKW), bf)
        for t in range(3):
            rb = qb ^ t
            xb[:, t, :] = xT[b][:, rb * BLKW : (rb + 1) * BLKW]
        in_maps.append(
            {
                "xb": np.ascontiguousarray(xb),
                "wqT": wqT,
                "wkT": wkT,
                "wvT": wvT,
                "rmsw": rw,
            }
        )
    return in_maps


def kernel(x, Wq, Wk, Wv, lambda_q1, lambda_q2, lambda_k1, lambda_k2, rms_weight):
    lq1 = np.asarray(lambda_q1, np.float32)
    lq2 = np.asarray(lambda_q2, np.float32)
    lk1 = np.asarray(lambda_k1, np.float32)
    lk2 = np.asarray(lambda_k2, np.float32)
    lam = float(
        np.exp(np.dot(lq1, lk1)) - np.exp(np.dot(lq2, lk2)) + LAMBDA_INIT
    )
    nc = build(lam)
    in_maps = make_in_maps(x, Wq, Wk, Wv, rms_weight)
    res = bass_utils.run_bass_kernel_spmd(nc, in_maps, core_ids=list(range(NCORES)))
    out = np.empty((B, S, H), np.float32)
    for core in range(NCORES):
        b, qb = divmod(core, NCORES // B)
        out[b, qb * QSHARD : (qb + 1) * QSHARD] = res.results[core]["out"]
    return out


# revision 38
# speedup vs baseline: 1.5859x; 1.0094x over previous
"""DiffAttn (differential attention) Trainium2 Bass kernel, v2.

Self-contained: kernel(**inputs) takes the FULL unsharded inputs as numpy
arrays and returns the FULL output [2, 4096, 128] float32.

Sharding: 8 cores = (batch in {0,1}) x (query-block of 1024 rows).  Key
blocks are consumed in XOR-delta order (softmax over keys is permutation
invariant, so block order is irrelevant): the host stages each core's x
slices pre-permuted as [own, own^1, own^2], the core projects K/V for those
three blocks locally, and block own^3 is exchanged through a tiny PAIRWISE
AllGather ([[0,3],[1,2],[4,7],[5,6]]) whose output holds {own, own^3} in
rank order -- both halves are attended (own is NOT attended from the local
projection), which keeps the program identity-free SPMD.

Scores are computed TRANSPOSED ([sk, sq], keys on partitions) so exp(scores)
feeds the PV matmul directly as the moving operand (attention contracts over
sk).  Softmax row-sums are NOT computed with per-chunk ones-matmuls (that
costs as much PE as the PV itself): instead the DVE accumulates e chunk-wise
into an fp16 running sum tile and a single ones-matmul per group recovers
the sums at the end of the sweep.  exp is biased by -EXP_BIAS (cancels in
u/sums) so the fp16 accumulation cannot overflow.

The main loop is ACT(exp)-bound; emission is software-pipelined depth-1
(scores(i+1) issues before PV(i)) which keeps the ACT engine saturated.
"""

import math
import os
import sys
from contextlib import ExitStack

import numpy as np

for _p in ("/root/.axon_site/_ro/trn_rl_repo", "/opt/trn_rl_repo"):
    if os.path.isdir(_p) and _p not in sys.path:
        sys.path.append(_p)

import ml_dtypes  # noqa: E402

import concourse.bass as bass  # noqa: E402
import concourse.mybir as mybir  # noqa: E402
import concourse.tile as tile  # noqa: E402
from concourse import bacc, bass_utils  # noqa: E402
from concourse.masks import make_identity  # noqa: E402

B, S, D, H = 2, 4096, 2048, 128
H2 = H // 2  # 64
P = 128
NCORES = 8
QSHARD = 1024  # q rows per core
DCH = D // P  # 16 d-chunks
NGROUPS, GW = 2, 512  # q groups per core
NBLK, BLKW = 4, 1024  # attended key blocks per core
BCH = BLKW // P  # 8 key chunks per block
NJ = GW // P  # 4 q sub-blocks of 128 per group

LAMBDA_INIT = 0.8 - 0.6 * math.exp(-0.3 * 12)
RMS_EPS = float(np.finfo(np.float32).eps)
SCALE = 1.0 / math.sqrt(H2)
EXP_BIAS = 4.0  # exp(s*SCALE - EXP_BIAS); cancels in u/sums

F32 = mybir.dt.float32
BF16 = mybir.dt.bfloat16
F16 = mybir.dt.float16

AF = mybir.ActivationFunctionType
OP = mybir.AluOpType


def _emit(ctx: ExitStack, tc: "tile.TileContext", lam: float):
    nc = tc.nc

    # x slices for blocks [own, own^1, own^2] (host pre-permutes per core)
    xb = nc.dram_tensor("xb", (D, 3, BLKW), BF16, kind="ExternalInput").ap()
    part_d = nc.dram_tensor("part_d", (2, P, BLKW), F16).ap()
    pair_d = nc.dram_tensor("pair_d", (4, P, BLKW), F16).ap()
    wqT = nc.dram_tensor("wqT", (D, H), BF16, kind="ExternalInput").ap()
    wkT = nc.dram_tensor("wkT", (D, H), BF16, kind="ExternalInput").ap()
    wvT = nc.dram_tensor("wvT", (D, H), BF16, kind="ExternalInput").ap()
    rmsw = nc.dram_tensor("rmsw", (H,), F32, kind="ExternalInput").ap()
    out_d = nc.dram_tensor("out", (QSHARD, H), F32, kind="ExternalOutput").ap()

    # ---- constant / persistent SBUF tiles ----
    consts = ctx.enter_context(tc.tile_pool(name="consts", bufs=1))
    persist = ctx.enter_context(tc.tile_pool(name="persist", bufs=1))

    ident = consts.tile([P, P], F32)
    make_identity(nc, ident)
    ident16 = consts.tile([P, P], F16)
    make_identity(nc, ident16)
    ones_f16 = consts.tile([P, 1], F16)
    nc.vector.memset(ones_f16, 1.0)
    nbias = consts.tile([P, 1], F32)
    nc.vector.memset(nbias, -EXP_BIAS)
    sq_warm = consts.tile([P, 1], F32)
    nc.scalar.activation(sq_warm, ones_f16, AF.Sqrt)
    rmsw_bc = consts.tile([P, H], F32)
    nc.sync.dma_start(
        out=rmsw_bc,
        in_=bass.AP(tensor=rmsw.tensor, offset=0, ap=[[0, P], [1, H]]),
    )
    wq_sb = consts.tile([P, DCH, H], BF16)
    wk_sb = consts.tile([P, DCH, H], BF16)
    wv_sb = consts.tile([P, DCH, H], BF16)

    qT_sb = persist.tile([P, QSHARD], F16)  # [h, sq]
    # attended key blocks: slot 0 = own^1, 1 = own^2, 2/3 = pair {own, own^3}
    kT_sb = persist.tile([P, NBLK, BLKW], F16)  # [h, blk, sk]
    v_sb = persist.tile([P, NBLK, BCH, P], F16)  # [sk%128, blk, chunk, h]
    kpart_sb = persist.tile([P, BLKW], F16)  # own k (collective input only)
    vpart_sb = persist.tile([P, BCH, P], F16)  # own v (collective input only)

    xpool = ctx.enter_context(tc.tile_pool(name="xstream", bufs=3))
    epool = ctx.enter_context(tc.tile_pool(name="epool", bufs=6))
    accp = ctx.enter_context(tc.tile_pool(name="accp", bufs=1))
    usb_pool = ctx.enter_context(tc.tile_pool(name="usb", bufs=2))
    small = ctx.enter_context(tc.tile_pool(name="small", bufs=8))
    outp = ctx.enter_context(tc.tile_pool(name="outp", bufs=4))
    attn_pool = ctx.enter_context(tc.tile_pool(name="attnp", bufs=2 * NGROUPS * NJ + 1))

    xb_r = xb.rearrange("(c p) t q -> p c t q", p=P)

    # x0 uploads first; x1/x2 are emitted AFTER the part DMAs + collective
    # so the pairwise exchange dispatches at ~18us instead of queueing
    # behind 23us of x traffic
    nc.sync.dma_start(out=wk_sb, in_=wkT.rearrange("(c p) h -> p c h", p=P))
    x_tiles = [
        xpool.tile([P, DCH, BLKW], BF16, tag="x", name=f"x{t}") for t in range(3)
    ]

    def upload_x(t, npc):
        step = DCH // npc
        for pc in range(npc):
            nc.sync.dma_start(
                out=x_tiles[t][:, pc * step : (pc + 1) * step, :],
                in_=xb_r[:, pc * step : (pc + 1) * step, t, :],
            )
            if t == 0 and pc == 0:
                nc.sync.dma_start(
                    out=wv_sb, in_=wvT.rearrange("(c p) h -> p c h", p=P)
                )
            if t == 0 and pc == 5:
                nc.sync.dma_start(
                    out=wq_sb, in_=wqT.rearrange("(c p) h -> p c h", p=P)
                )

    upload_x(0, 16)

    vT_scr = persist.tile([P, BLKW], F16)  # scratch: vT before transposing

    def proj_par(pool, x_sb, kdst, vdst):
        """4-bank parallel c-outer K + vT projection chasing the x pieces."""
        ks = [
            pool.tile([P, GW], F32, tag=f"kacc{i}", bufs=1, name=f"kacc{i}")
            for i in range(2)
        ]
        vs = [
            pool.tile([P, GW], F32, tag=f"vacc{i}", bufs=1, name=f"vacc{i}")
            for i in range(2)
        ]
        for c in range(DCH):
            for sl in range(2):
                xs = x_sb[:, c, sl * GW : (sl + 1) * GW]
                nc.tensor.matmul(
                    ks[sl], wk_sb[:, c, :], xs, start=(c == 0), stop=(c == DCH - 1)
                )
                nc.tensor.matmul(
                    vs[sl], wv_sb[:, c, :], xs, start=(c == 0), stop=(c == DCH - 1)
                )
        for sl in range(2):
            nc.scalar.copy(kdst[:, sl * GW : (sl + 1) * GW], ks[sl])
            nc.scalar.copy(vT_scr[:, sl * GW : (sl + 1) * GW], vs[sl])
        for j in range(BCH):
            tp = pool.tile([P, P], F16, tag=f"vacc{j % 2}", bufs=1, name=f"tp{j % 2}")
            nc.tensor.transpose(tp, vT_scr[:, j * P : (j + 1) * P], ident16)
            nc.vector.tensor_copy(vdst[:, j, :], tp)

    def proj_quanta(pool, x_sb, kdst, vdst):
        """2-bank serialized projection as emission quanta, interleaved with
        the group-0 attention sweep."""
        q = []
        ks = [
            pool.tile([P, GW], F32, tag=f"pk{i}", bufs=1, name=f"pk{i}")
            for i in range(2)
        ]

        def kmm(c, sl):
            nc.tensor.matmul(
                ks[sl],
                wk_sb[:, c, :],
                x_sb[:, c, sl * GW : (sl + 1) * GW],
                start=(c == 0),
                stop=(c == DCH - 1),
            )

        for c in range(DCH):
            q.append(lambda c=c: (kmm(c, 0), kmm(c, 1)))
        q.append(
            lambda: [
                nc.scalar.copy(kdst[:, sl * GW : (sl + 1) * GW], ks[sl])
                for sl in range(2)
            ]
        )
        vst = {}

        def valloc():
            vst[0] = pool.tile([P, GW], F32, tag="pk0", bufs=1, name="pv0")
            vst[1] = pool.tile([P, GW], F32, tag="pk1", bufs=1, name="pv1")

        def vmm(c, sl):
            nc.tensor.matmul(
                vst[sl],
                wv_sb[:, c, :],
                x_sb[:, c, sl * GW : (sl + 1) * GW],
                start=(c == 0),
                stop=(c == DCH - 1),
            )

        q.append(valloc)
        for c in range(DCH):
            q.append(lambda c=c: (vmm(c, 0), vmm(c, 1)))
        q.append(
            lambda: [
                nc.scalar.copy(vT_scr[:, sl * GW : (sl + 1) * GW], vst[sl])
                for sl in range(2)
            ]
        )

        def tpj(j):
            tp = pool.tile([P, P], F16, tag=f"pk{j % 2}", bufs=1, name=f"ptp{j % 2}")
            nc.tensor.transpose(tp, vT_scr[:, j * P : (j + 1) * P], ident16)
            nc.vector.tensor_copy(vdst[:, j, :], tp)

        for j in range(BCH):
            q.append(lambda j=j: tpj(j))
        return q

    # ---- phase A: own + d1 projections (4-bank pool, closed after) ----
    with tc.tile_pool(name="pp_proj4", space="PSUM", bufs=1) as pp_proj4:
        for w in range(48):
            wtp = pp_proj4.tile(
                [P, P], F16, tag=f"vacc{w % 2}", bufs=1, name=f"tp{w % 2}"
            )
            nc.tensor.transpose(wtp, ident16, ident16)
        proj_par(pp_proj4, x_tiles[0], kpart_sb, vpart_sb)
        nc.sync.dma_start(out=part_d[0], in_=kpart_sb)
        nc.sync.dma_start(out=part_d[1], in_=vpart_sb.rearrange("p j h -> p (j h)"))
        nc.gpsimd.collective_compute(
            "AllGather",
            OP.bypass,
            replica_groups=[[0, 3], [1, 2], [4, 7], [5, 6]],
            ins=[part_d.opt()],
            outs=[pair_d.opt()],
        )
        upload_x(1, 8)
        upload_x(2, 8)
        # group-0 half of q only; g1's half projects inside the g0 sweep
        qacc = pp_proj4.tile([P, GW], F32, tag="kacc0", bufs=1, name="qacc")
        for c in range(DCH):
            nc.tensor.matmul(
                qacc,
                wq_sb[:, c, :],
                x_tiles[0][:, c, 0:GW],
                start=(c == 0),
                stop=(c == DCH - 1),
            )
        nc.scalar.copy(qT_sb[:, 0:GW], qacc)
        proj_par(pp_proj4, x_tiles[1], kT_sb[:, 0, :], v_sb[:, 0, :, :])

    # unpack the gathered pair {own, own^3} into slots 2, 3
    for r in range(2):
        nc.sync.dma_start(out=kT_sb[:, 2 + r, :], in_=pair_d[2 * r])
        nc.sync.dma_start(
            out=v_sb[:, 2 + r, :, :],
            in_=pair_d[2 * r + 1].rearrange("p (j h) -> p j h", j=BCH),
        )

    # ---- g-major sweeps: only one u accumulator live at a time, which
    # frees 2 PSUM banks during g0's sweep for the in-loop d2 projection ----
    pp_s = ctx.enter_context(tc.tile_pool(name="pp_s", space="PSUM", bufs=2))
    pp_u0 = ctx.enter_context(tc.tile_pool(name="pp_u0", space="PSUM", bufs=1))
    pools_u = {0: pp_u0}
    u_ps = {0: pp_u0.tile([P, 2, GW], F32, tag="u0", bufs=1, name="u0")}
    acc = [accp.tile([P, 2 * GW], F16, tag=f"acc{g}", name=f"acc{g}") for g in range(2)]

    pending = None  # (e_sb, g, slot, ch, first, last)

    def flush_pending():
        nonlocal pending
        if pending is None:
            return
        e_sb, g, slot, ch, first, last = pending
        for hf in range(2):
            nc.tensor.matmul(
                u_ps[g][:, hf, :],
                v_sb[:, slot, ch, :],
                e_sb[:, hf * GW : (hf + 1) * GW],
                start=first,
                stop=last,
            )
        if first:
            nc.vector.tensor_copy(acc[g], e_sb)
        else:
            nc.vector.tensor_tensor(acc[g], acc[g], e_sb, op=OP.add)
        pending = None

    def attend(g, slot, ch, first, last):
        nonlocal pending
        q0 = g * GW
        s_ps = pp_s.tile([P, 2 * GW], F32, tag="s", name="s_ps")
        nc.tensor.matmul(
            s_ps[:, 0:GW],
            kT_sb[0:H2, slot, ch * P : (ch + 1) * P],
            qT_sb[0:H2, q0 : q0 + GW],
        )
        nc.tensor.matmul(
            s_ps[:, GW : 2 * GW],
            kT_sb[H2:H, slot, ch * P : (ch + 1) * P],
            qT_sb[H2:H, q0 : q0 + GW],
        )
        e_sb = epool.tile([P, 2 * GW], F16, tag="e", name="e_sb")
        nc.scalar.activation(e_sb, s_ps, AF.Exp, scale=SCALE, bias=nbias)
        flush_pending()
        pending = (e_sb, g, slot, ch, first, last)

    c_ = 1.0 - LAMBDA_INIT
    a_ = 1.0 / (H * c_ * c_)
    b_ = RMS_EPS / (c_ * c_)
    sums_acc = small.tile([1, NGROUPS, 2 * GW], F32, tag="sums_acc", bufs=1)
    r_sb = small.tile([P, 2 * 2 * NJ], F32, tag="r", bufs=1)
    rmsall = small.tile([P, NGROUPS * NJ], F32, tag="rmsall", bufs=1)
    u_sbs = {}
    attns_all = {}

    def group_tail(g, use_s=False):
        tpool = pp_s if use_s else pools_u[g]
        ttag = "s" if use_s else f"u{g}"
        tbufs = 2 if use_s else 1
        """u -> SBUF, sums, normalization factors, combine, RMS, store —
        emitted right after group g's final PV so g0's tail overlaps g1's
        last block of attention."""
        u_sb = usb_pool.tile([P, 2 * GW], F32, tag="usb")
        nc.vector.tensor_copy(u_sb, u_ps[g][:, :, :])
        u_sbs[g] = u_sb
        sums_ps = tpool.tile([P, 2 * GW], F32, tag=ttag, bufs=tbufs, name=f"sums{g}")
        for hf in range(2):
            sl = slice(hf * GW, (hf + 1) * GW)
            nc.tensor.matmul(sums_ps[0:1, sl], ones_f16, acc[g][:, sl])
        nc.vector.tensor_copy(sums_acc[0:1, g, :], sums_ps[0:1, :])
        sumsT_ps = tpool.tile([P, 2 * NJ], F32, tag=ttag, bufs=tbufs, name=f"sumsT{g}")
        for hf in range(2):
            for j in range(NJ):
                nc.tensor.transpose(
                    sumsT_ps[:, hf * NJ + j : hf * NJ + j + 1],
                    sums_acc[0:1, g, hf * GW + j * P : hf * GW + (j + 1) * P],
                    ident[0:1, 0:1],
                )
        rg = r_sb[:, g * 2 * NJ : (g + 1) * 2 * NJ]
        nc.vector.reciprocal(rg, sumsT_ps)
        nc.vector.tensor_scalar_mul(
            r_sb[:, g * 2 * NJ + NJ : (g + 1) * 2 * NJ],
            r_sb[:, g * 2 * NJ + NJ : (g + 1) * 2 * NJ],
            lam,
        )
        post_ps = tpool.tile([P, 2 * NJ, P], F32, tag=ttag, bufs=tbufs, name=f"post{g}")
        for j in range(NJ):
            nc.tensor.transpose(
                post_ps[:, j, :], u_sb[:, j * P : (j + 1) * P], ident
            )
            nc.tensor.transpose(
                post_ps[:, NJ + j, :], u_sb[:, GW + j * P : GW + (j + 1) * P], ident
            )
        attns = []
        for j in range(NJ):
            rcol = g * 2 * NJ
            idx = g * NJ + j
            t2 = small.tile([P, P], F32, tag="t2")
            nc.vector.scalar_tensor_tensor(
                t2,
                post_ps[:, NJ + j, :],
                r_sb[:, rcol + NJ + j : rcol + NJ + j + 1],
                rmsw_bc,
                op0=OP.mult,
                op1=OP.bypass,
            )
            attn_sb = attn_pool.tile([P, P], F32, tag="attn")
            nc.vector.scalar_tensor_tensor(
                attn_sb,
                post_ps[:, j, :],
                r_sb[:, rcol + j : rcol + j + 1],
                t2,
                op0=OP.mult,
                op1=OP.subtract,
            )
            sq_scr = small.tile([P, P], F32, tag="sqscr")
            # scale folds 1/(H*c^2) into the squares; the eps term is 5
            # orders below the signal and is dropped
            nc.scalar.activation(
                sq_scr,
                attn_sb,
                AF.Square,
                scale=math.sqrt(a_),
                accum_out=rmsall[:, idx : idx + 1],
            )
            attns.append(attn_sb)
        attns_all[g] = attns

    def final_tail():
        # ONE Sqrt for both groups: avoids a second Exp<->Sqrt ACT-table
        # switch inside the ACT-bound sweep
        roots = small.tile([P, NGROUPS * NJ], F32, tag="roots", bufs=1)
        nc.scalar.activation(roots, rmsall, AF.Sqrt)
        rrms = small.tile([P, NGROUPS * NJ], F32, tag="rrms", bufs=1)
        nc.vector.reciprocal(rrms, roots)
        o_all = outp.tile([P, NGROUPS * NJ, H], F32, tag="o", bufs=1, name="o")
        for g in range(NGROUPS):
            for j in range(NJ):
                idx = g * NJ + j
                nc.vector.scalar_tensor_tensor(
                    o_all[:, idx, :],
                    attns_all[g][j],
                    rrms[:, idx : idx + 1],
                    rmsw_bc,
                    op0=OP.mult,
                    op1=OP.mult,
                )
        nc.sync.dma_start(
            out=out_d.rearrange("(j p) h -> p j h", p=P), in_=o_all
        )

    # ---- phase B: g0 sweeps all 4 blocks; d2 + q(g1) project in the
    # PE slack using the 2 banks that u1 does not occupy yet ----
    with tc.tile_pool(name="pp_proj2", space="PSUM", bufs=1) as pp_proj2:
        quanta = proj_quanta(pp_proj2, x_tiles[2], kT_sb[:, 1, :], v_sb[:, 1, :, :])

        def qg1_quanta():
            q = []
            st = {}

            def alloc():
                st[0] = pp_proj2.tile([P, GW], F32, tag="pk0", bufs=1, name="qg1")

            q.append(alloc)

            def qmm(c):
                nc.tensor.matmul(
                    st[0],
                    wq_sb[:, c, :],
                    x_tiles[0][:, c, GW : 2 * GW],
                    start=(c == 0),
                    stop=(c == DCH - 1),
                )

            for c in range(DCH):
                q.append(lambda c=c: qmm(c))
            q.append(lambda: nc.scalar.copy(qT_sb[:, GW : 2 * GW], st[0]))
            return q

        quanta += qg1_quanta()
        qi = 0
        seq0 = [(s, c) for s in (0, 1, 2, 3) for c in range(BCH)]
        for i, (slot, ch) in enumerate(seq0):
            attend(0, slot, ch, first=(i == 0), last=(i == len(seq0) - 1))
            # d2's projection must land before its first attend (i == 8):
            # drain most quanta across the first 8 chunks, the rest after
            budget = 8 if i < 7 else len(quanta)
            took = 0
            while qi < len(quanta) and took < budget:
                quanta[qi]()
                qi += 1
                took += 1
        flush_pending()

    # ---- phase C: u1 takes the banks proj2 released; g0's tail overlaps ----
    pp_u1 = ctx.enter_context(tc.tile_pool(name="pp_u1", space="PSUM", bufs=1))
    pools_u[1] = pp_u1
    u_ps[1] = pp_u1.tile([P, 2, GW], F32, tag="u1", bufs=1, name="u1")
    seq1 = [(s, c) for s in range(NBLK) for c in range(BCH)]
    for i, (slot, ch) in enumerate(seq1):
        attend(1, slot, ch, first=(i == 0), last=(i == len(seq1) - 1))
        if i == 1:
            group_tail(0)
    flush_pending()
    group_tail(1, use_s=True)
    final_tail()


def build(lam: float):
    from concourse._compat import axon_active

    nc = bacc.Bacc(
        "TRN2",
        target_bir_lowering=False,
        debug=not axon_active(),
        num_devices=NCORES,
    )
    with tile.TileContext(nc) as tc:
        with ExitStack() as ctx:
            _emit(ctx, tc, lam)
    nc.compile()
    return nc


def make_in_maps(x, Wq, Wk, Wv, rms_weight):
    bf = ml_dtypes.bfloat16
    x = np.asarray(x, dtype=np.float32)
    xT = np.ascontiguousarray(x.transpose(0, 2, 1)).astype(bf)  # [B, D, S]
    wqT = np.ascontiguousarray(np.asarray(Wq, np.float32).T).astype(bf)
    wkT = np.ascontiguousarray(np.asarray(Wk, np.float32).T).astype(bf)
    wvT = np.ascontiguousarray(np.asarray(Wv, np.float32).T).astype(bf)
    rw = np.ascontiguousarray(np.asarray(rms_weight, np.float32))
    in_maps = []
    for core in range(NCORES):
        b, qb = divmod(core, NCORES // B)
        xb = np.empty((D, 3, BLKW), bf)
        for t in range(3):
            rb = qb ^ t
            xb[:, t, :] = xT[b][:, rb * BLKW : (rb + 1) * BLKW]
        in_maps.append(
            {
                "xb": np.ascontiguousarray(xb),
                "wqT": wqT,
                "wkT": wkT,
                "wvT": wvT,
                "rmsw": rw,
            }
        )
    return in_maps


def kernel(x, Wq, Wk, Wv, lambda_q1, lambda_q2, lambda_k1, lambda_k2, rms_weight):
    lq1 = np.asarray(lambda_q1, np.float32)
    lq2 = np.asarray(lambda_q2, np.float32)
    lk1 = np.asarray(lambda_k1, np.float32)
    lk2 = np.asarray(lambda_k2, np.float32)
    lam = float(
        np.exp(np.dot(lq1, lk1)) - np.exp(np.dot(lq2, lk2)) + LAMBDA_INIT
    )
    nc = build(lam)
    in_maps = make_in_maps(x, Wq, Wk, Wv, rms_weight)
    res = bass_utils.run_bass_kernel_spmd(nc, in_maps, core_ids=list(range(NCORES)))
    out = np.empty((B, S, H), np.float32)
    for core in range(NCORES):
        b, qb = divmod(core, NCORES // B)
        out[b, qb * QSHARD : (qb + 1) * QSHARD] = res.results[core]["out"]
    return out
